# revision 1
# baseline (speedup 1.0000x reference)
"""Trainium2 Bass kernel for a custom attention block (qkv-proj + LN(q,k) +
RoPE + causal attention + out-proj), distributed over 8 NeuronCores.

Sharding: 2 cores per batch (B=4). Core role r=c%2 takes q-token blocks
{0,3} (r=0) or {1,2} (r=1) of 512 tokens; every core computes K/V for the
full 2048-token sequence of its batch (no collectives). The compiled
program is identical on all cores; all per-core differences are input
data (sliced x^T, cos/sin tables, causal masks).

Orientation: q^T / k^T are produced feature-on-partition ([hd, tokens]),
v token-on-partition. Attention computes s^T = (k^T)^T-slice @ q^T with
fp32r matmuls (full PE rate at moving-dim >= 256), exp(s - 8), mask
multiply (data-driven), PV as lhsT=v rhs=exp -> out^T, denominator via a
ones-column matmul, so no on-chip transposes are needed anywhere.

LN: mean subtraction is folded into host-pre-centered w_in rows; variance
comes from Square + ones-matmul partition reduction; rsqrt(var+eps) is
computed as Exp(-0.5*Ln(var+eps)) so all ACT functions live in one table
set (natural_log_exp_and_others).
"""

import math

import numpy as np

import concourse.bass as bass
import concourse.mybir as mybir
import concourse.tile as tile
from concourse import bacc
from concourse.bass import ds, ts

F32 = mybir.dt.float32
F32R = mybir.dt.float32r
AF = mybir.ActivationFunctionType
OP = mybir.AluOpType

P = 128
HD = 128

FULL_CFG = dict(
    D=2048,           # model dim (contraction dim for projections)
    S=2048,           # kv tokens per core (full sequence of its batch)
    NQTOK=1024,       # q tokens per core
    PT=256,           # projection s-tile width (moving dim)
    QT=512,           # attention q-tile width (moving dim)
    slots=(8, 16),    # kv 128-chunks visited per q-tile
    masked=(tuple(range(0, 8)), tuple(range(8, 16))),  # slots that get a mask
    EXP_BIAS=8.0,
    EPS=1e-5,
    MASK_F32=False,
)

SMALL_CFG = dict(
    D=512,
    S=1024,
    NQTOK=1024,
    PT=256,
    QT=512,
    slots=(8, 8),
    masked=(tuple(range(0, 8)), tuple(range(4, 8))),
    EXP_BIAS=8.0,
    EPS=1e-5,
    MASK_F32=False,
)


def _r(ap):
    """fp32 -> fp32r view for matmul operands."""
    return ap.bitcast(F32R)


def build_program(cfg):
    D = cfg["D"]
    S = cfg["S"]
    NQTOK = cfg["NQTOK"]
    PT = cfg["PT"]
    QT = cfg["QT"]
    slots = cfg["slots"]
    masked = cfg["masked"]
    EXP_BIAS = cfg["EXP_BIAS"]
    EPS = cfg["EPS"]

    NH = D // HD              # heads == e-chunks per q (and per k)
    DC = D // P               # contraction chunks
    NQ = NQTOK // QT          # q tiles
    S2 = S // 2               # kv half (x residency granularity)
    KC2 = S2 // P             # kv chunks per half
    VET = max(1, D // 512)    # v e-tiles of width 512
    VEW = min(512, D)         # v e-tile width
    VH = VEW // HD            # heads per v e-tile
    MAXM = max(len(m) for m in masked)
    QST_PER_TILE = QT // PT
    mdt = F32 if cfg.get("MASK_F32", True) else mybir.dt.bfloat16

    nc = bacc.Bacc("TRN2", target_bir_lowering=False, debug=False)

    # ---- I/O ----
    xTq = nc.dram_tensor("xTq", [D, NQTOK], F32, kind="ExternalInput").ap()
    xT = nc.dram_tensor("xT", [D, S], F32, kind="ExternalInput").ap()
    wqkT = nc.dram_tensor("wqkT", [2 * NH, P, DC, P], F32,
                          kind="ExternalInput").ap()
    wvT = nc.dram_tensor("wvT", [D, D], F32, kind="ExternalInput").ap()
    woT = nc.dram_tensor("woT", [D, D], F32, kind="ExternalInput").ap()
    cosq_i = nc.dram_tensor("cosq", [HD, NQTOK], F32, kind="ExternalInput").ap()
    sinq_i = nc.dram_tensor("sinqn", [HD, NQTOK], F32, kind="ExternalInput").ap()
    cosk_i = nc.dram_tensor("cosk", [HD, S], F32, kind="ExternalInput").ap()
    sink_i = nc.dram_tensor("sinkn", [HD, S], F32, kind="ExternalInput").ap()
    rotm_i = nc.dram_tensor("rotm", [P, P], F32, kind="ExternalInput").ap()
    onesc_i = nc.dram_tensor("onesc", [P, 1], F32, kind="ExternalInput").ap()
    onesr_i = nc.dram_tensor("onesr", [1, P], F32, kind="ExternalInput").ap()
    gq_i = nc.dram_tensor("gq", [P, NH], F32, kind="ExternalInput").ap()
    bq_i = nc.dram_tensor("bq", [P, NH], F32, kind="ExternalInput").ap()
    gk_i = nc.dram_tensor("gk", [P, NH], F32, kind="ExternalInput").ap()
    bk_i = nc.dram_tensor("bk", [P, NH], F32, kind="ExternalInput").ap()
    masks_i = nc.dram_tensor("masks", [NQ, P, MAXM, QT], mdt, kind="ExternalInput").ap()
    out_t = nc.dram_tensor("out", [D, NQTOK], F32, kind="ExternalOutput").ap()

    with tile.TileContext(nc) as tc:
        import contextlib

        ctx = contextlib.ExitStack()
        with ctx:
            sb = ctx.enter_context(tc.tile_pool(name="sb", bufs=1))
            psum = ctx.enter_context(tc.tile_pool(name="ps", bufs=1, space="PSUM"))
            dram = ctx.enter_context(tc.tile_pool(name="dram", bufs=1, space="DRAM"))

            # ---- DRAM scratch ----
            qts = dram.tile([P, NH, NQTOK], F32, tag="qts", name="qts")
            kts = dram.tile([P, NH, S], F32, tag="kts", name="kts")
            vs = dram.tile([NH, S, HD], F32, tag="vs", name="vs")
            ots = dram.tile([P, NH, NQTOK], F32, tag="ots", name="ots")

            # ---- constants / small inputs ----
            ones_col = sb.tile([P, 1], F32, tag="ones_col", name="ones_col")
            nc.sync.dma_start(_r(ones_col), _r(onesc_i))
            ones_row = sb.tile([1, P], F32, tag="ones_row", name="ones_row")
            nc.sync.dma_start(_r(ones_row), _r(onesr_i))
            eps1 = sb.tile([1, 1], F32, tag="eps1", name="eps1")
            nc.vector.memset(eps1, EPS)
            zero1 = sb.tile([1, 1], F32, tag="zero1", name="zero1")
            nc.vector.memset(zero1, 0.0)
            nege = sb.tile([P, 1], F32, tag="nege", name="nege")
            nc.vector.memset(nege, -EXP_BIAS)
            rotm = sb.tile([P, P], F32, tag="rotm", name="rotm")
            nc.sync.dma_start(_r(rotm), _r(rotm_i))
            gq = sb.tile([P, NH], F32, tag="gq", name="gq")
            nc.sync.dma_start(gq, gq_i)
            bq = sb.tile([P, NH], F32, tag="bq", name="bq")
            nc.sync.dma_start(bq, bq_i)
            gk = sb.tile([P, NH], F32, tag="gk", name="gk")
            nc.sync.dma_start(gk, gk_i)
            bk = sb.tile([P, NH], F32, tag="bk", name="bk")
            nc.sync.dma_start(bk, bk_i)

            def proj_ln_rope(x_sb, n_tok, st_global_off, wcol_off, n_st,
                             cos_sb, sin_sb, g_sb, b_sb, dst, tok0):
                """Project x_sb -> feature-partition [e, s] tiles, LN, rope,
                write to dst[:, :, tok0 + st*PT ...].

                x_sb: [P, DC, n_tok] sbuf; st covers n_st tiles of PT inside.
                wcol_off: column offset into wqkT (0 for q, D for k).
                cos_sb/sin_sb indexed at st_global_off + local offsets.
                """
                assert n_st % 2 == 0
                for grp in range(n_st // 2):
                    sts = [grp * 2, grp * 2 + 1]
                    gsl = ds(st_global_off + grp * 2 * PT, 2 * PT)
                    cos_t = sb.tile([HD, 2 * PT], F32, tag="cos", bufs=2,
                                    name="cos_t")
                    nc.sync.dma_start(cos_t, cos_sb[:, gsl])
                    sin_t = sb.tile([HD, 2 * PT], F32, tag="sin", bufs=2,
                                    name="sin_t")
                    nc.sync.dma_start(sin_t, sin_sb[:, gsl])
                    holds = {}
                    pstats = {}
                    for st in sts:
                        holds[st] = sb.tile([P, NH, PT], F32, tag="hold",
                                            bufs=3, name="hold")
                        pstats[st] = psum.tile([1, PT], F32, tag="stat",
                                               bufs=4, name="ps_stat")
                    for ec in range(NH):
                        w = sb.tile([P, DC, P], F32, tag="w", bufs=3, name="w")
                        nc.sync.dma_start(
                            _r(w), _r(wqkT[wcol_off // P + ec])
                        )
                        pss = {st: psum.tile([P, PT], F32, tag="mm", bufs=4,
                                             name="ps")
                               for st in sts}
                        for d in range(DC):
                            for st in sts:
                                nc.tensor.matmul(
                                    pss[st],
                                    lhsT=_r(w[:, d]),
                                    rhs=_r(x_sb[:, d, ds(st * PT, PT)]),
                                    start=(d == 0),
                                    stop=(d == DC - 1),
                                )
                        for st in sts:
                            nc.vector.tensor_copy(_r(holds[st][:, ec]), pss[st])
                            sq = sb.tile([P, PT], F32, tag="sq", bufs=2,
                                         name="sq")
                            nc.scalar.square(_r(sq), pss[st])
                            nc.tensor.matmul(
                                pstats[st],
                                lhsT=_r(ones_col),
                                rhs=_r(sq),
                                start=(ec == 0),
                                stop=(ec == NH - 1),
                            )
                    for st in sts:
                        hold = holds[st]
                        csl = ds((st % 2) * PT, PT)
                        # rsig = exp(-0.5 * ln(sumsq/D + eps))
                        lnv = sb.tile([1, PT], F32, tag="stats_sb", bufs=4,
                                      name="lnv")
                        nc.scalar.activation(lnv, pstats[st], AF.Ln,
                                             scale=1.0 / D, bias=eps1)
                        rsig = sb.tile([1, PT], F32, tag="stats_sb", bufs=4,
                                       name="rsig")
                        nc.scalar.activation(_r(rsig), lnv, AF.Exp, bias=zero1,
                                             scale=-0.5)
                        ps_rep = psum.tile([P, PT], F32, tag="stat", bufs=4,
                                           name="ps_rep")
                        nc.tensor.matmul(ps_rep, lhsT=_r(ones_row),
                                         rhs=_r(rsig))
                        # pass 1: DVE LN apply for all chunks first, so the
                        # rotation matmuls never head-of-line block the
                        # in-order PE stream on a DVE dependency.
                        for ec in range(NH):
                            ch = hold[:, ec]
                            nc.vector.tensor_tensor(_r(ch), ch, ps_rep,
                                                    op=OP.mult)
                            nc.vector.tensor_scalar(
                                _r(ch), ch,
                                scalar1=g_sb[:, ds(ec, 1)],
                                scalar2=b_sb[:, ds(ec, 1)],
                                op0=OP.mult, op1=OP.add,
                            )
                        # pass 2: rotation matmuls stream back-to-back
                        for ec in range(NH):
                            ch = hold[:, ec]
                            ps_rot = psum.tile([P, PT], F32, tag="mm", bufs=4,
                                               name="ps_rot")
                            nc.tensor.matmul(ps_rot, lhsT=_r(rotm), rhs=_r(ch))
                            tmp = sb.tile([P, PT], F32, tag="tmp", bufs=3,
                                          name="rtmp")
                            nc.vector.tensor_tensor(
                                tmp, ps_rot, sin_t[:, csl], op=OP.mult
                            )
                            nc.vector.tensor_tensor(_r(ch), ch, cos_t[:, csl],
                                                    op=OP.mult)
                            nc.vector.tensor_tensor(_r(ch), ch, tmp, op=OP.add)
                        nc.sync.dma_start(dst[:, :, ds(tok0 + st * PT, PT)],
                                          hold)

            # ---- Phase A: q projection ----
            xq = sb.tile([P, DC, max(NQTOK, S2)], F32, tag="bigx", bufs=1,
                         name="xq")
            xq = xq[:, :, :NQTOK]
            for d in range(DC):
                nc.sync.dma_start(_r(xq[:, d]), _r(xTq[ds(d * P, P), :]))
            proj_ln_rope(xq, NQTOK, 0, 0, NQTOK // PT, cosq_i, sinq_i,
                         gq, bq, qts, 0)

            # ---- Phase B+C: k and v projections, per x-half ----
            for half in range(2):
                xk = sb.tile([P, DC, max(NQTOK, S2)], F32, tag="bigx", bufs=1,
                             name="xk")
                xk = xk[:, :, :S2]
                for d in range(DC):
                    nc.sync.dma_start(
                        _r(xk[:, d]), _r(xT[ds(d * P, P), ds(half * S2, S2)])
                    )
                proj_ln_rope(xk, S2, half * S2, D, S2 // PT, cosk_i, sink_i,
                             gk, bk, kts, half * S2)
                # v: natural orientation, x as stationary
                n_grp = (KC2 + 3) // 4
                for grp in range(n_grp):
                    scs = [sc for sc in range(grp * 4, min((grp + 1) * 4, KC2))]
                    for et in range(VET):
                        psv = {}
                        for sc in scs:
                            psv[sc] = psum.tile([P, VEW], F32, tag="mm",
                                                bufs=4, name="psv")
                        for d in range(DC):
                            wv = sb.tile([P, VEW], F32, tag="w", bufs=3,
                                         name="wv")
                            nc.sync.dma_start(
                                _r(wv), _r(wvT[ds(d * P, P), ds(et * VEW, VEW)])
                            )
                            for sc in scs:
                                nc.tensor.matmul(
                                    psv[sc],
                                    lhsT=_r(xk[:, d, ds(sc * P, P)]),
                                    rhs=_r(wv),
                                    start=(d == 0),
                                    stop=(d == DC - 1),
                                )
                        for sc in scs:
                            vsb = sb.tile([P, VEW], F32, tag="vsb", bufs=2,
                                          name="vsb")
                            nc.vector.tensor_copy(vsb, psv[sc])
                            gsc = half * KC2 + sc
                            for hh in range(VH):
                                nc.sync.dma_start(
                                    vs[et * VH + hh, ds(gsc * P, P), :],
                                    vsb[:, ds(hh * HD, HD)],
                                )

            # ---- Phase D: attention + normalization ----
            for t in range(NQ):
                qsl_off = t * QT
                mt = sb.tile([P, MAXM, QT], mdt, tag="masks", bufs=1,
                             name="mt")
                nc.sync.dma_start(mt, masks_i[t])
                mpos = {kc: i for i, kc in enumerate(masked[t])}
                n_slots = slots[t]
                n_half = (n_slots + KC2 - 1) // KC2  # halves needed
                for h in range(NH):
                    qsl = sb.tile([P, QT], F32, tag="qslab", bufs=2,
                                  name="qsl")
                    nc.sync.dma_start(_r(qsl), _r(qts[:, h, ds(qsl_off, QT)]))
                    ksl = {}
                    vsl = {}
                    for hf in range(n_half):
                        ksl[hf] = sb.tile([P, S2], F32, tag="kslab", bufs=2,
                                          name="ksl")
                        nc.sync.dma_start(_r(ksl[hf]), _r(kts[:, h, ds(hf * S2, S2)]))
                        vsl[hf] = sb.tile([P, KC2, HD], F32, tag="vslab",
                                          bufs=2, name="vsl")
                        nc.sync.dma_start(
                            _r(vsl[hf]),
                            _r(vs[h, ds(hf * S2, S2), :].rearrange(
                                "(kc p) hd -> p kc hd", p=P
                            )),
                        )
                    psout = psum.tile([P, QT], F32, tag="mm", bufs=4,
                                      name="psout")
                    psden = psum.tile([1, QT], F32, tag="stat", bufs=4,
                                      name="psden")
                    for slot in range(n_slots):
                        hf = slot // KC2
                        kc = slot % KC2
                        pss = psum.tile([P, QT], F32, tag="mm", bufs=4,
                                        name="pss")
                        nc.tensor.matmul(
                            pss,
                            lhsT=_r(ksl[hf][:, ds(kc * P, P)]),
                            rhs=_r(qsl),
                        )
                        et = sb.tile([P, QT], F32, tag="exp", bufs=3,
                                     name="et")
                        nc.scalar.activation(_r(et), pss, AF.Exp, bias=nege)
                        if slot in mpos:
                            nc.vector.tensor_tensor(
                                _r(et), et, mt[:, mpos[slot]], op=OP.mult
                            )
                        nc.tensor.matmul(
                            psout,
                            lhsT=_r(vsl[hf][:, kc]),
                            rhs=_r(et),
                            start=(slot == 0),
                            stop=(slot == n_slots - 1),
                        )
                        nc.tensor.matmul(
                            psden,
                            lhsT=_r(ones_col),
                            rhs=_r(et),
                            start=(slot == 0),
                            stop=(slot == n_slots - 1),
                        )
                    rec0 = sb.tile([1, QT], F32, tag="stats_sb", bufs=4,
                                   name="rec0")
                    with nc.allow_low_precision(
                        reason="denominator reciprocal, 18 bits is plenty"
                    ):
                        nc.vector.reciprocal_approx_fast(rec0, psden)
                    rec = sb.tile([1, QT], F32, tag="stats_sb", bufs=4,
                                  name="rec")
                    nc.scalar.activation(_r(rec), rec0, AF.Copy)
                    psr = psum.tile([P, QT], F32, tag="stat", bufs=4,
                                    name="psr")
                    nc.tensor.matmul(psr, lhsT=_r(ones_row), rhs=_r(rec))
                    rsb = sb.tile([P, QT], F32, tag="tmp", bufs=3, name="rsb")
                    nc.scalar.activation(rsb, psr, AF.Copy)
                    ot = sb.tile([P, QT], F32, tag="outT", bufs=2, name="ot")
                    nc.vector.tensor_tensor(ot, psout, rsb, op=OP.mult)
                    nc.sync.dma_start(ots[:, h, ds(qsl_off, QT)], ot)

                # ---- Phase E: out-projection for this q tile ----
                EG = 4
                for eg in range(NH // EG):
                    psf = [
                        psum.tile([P, QT], F32, tag="mm", bufs=4, name="psf")
                        for _ in range(EG)
                    ]
                    for h in range(NH):
                        orh = sb.tile([P, QT], F32, tag="orhs", bufs=2,
                                      name="orh")
                        nc.sync.dma_start(_r(orh), _r(ots[:, h, ds(qsl_off, QT)]))
                        wo = sb.tile([P, EG * P], F32, tag="w", bufs=3,
                                     name="wo")
                        nc.sync.dma_start(
                            _r(wo), _r(woT[ds(h * P, P), ds(eg * EG * P, EG * P)])
                        )
                        for x in range(EG):
                            nc.tensor.matmul(
                                psf[x],
                                lhsT=_r(wo[:, ds(x * P, P)]),
                                rhs=_r(orh),
                                start=(h == 0),
                                stop=(h == NH - 1),
                            )
                    for x in range(EG):
                        fsb = sb.tile([P, QT], F32, tag="tmp", bufs=3,
                                      name="fsb")
                        nc.vector.tensor_copy(fsb, psf[x])
                        nc.sync.dma_start(
                            out_t[ds((eg * EG + x) * P, P), ds(qsl_off, QT)],
                            fsb,
                        )

    nc.compile()
    return nc


# --------------------------------------------------------------------------
# Host-side prep and driver
# --------------------------------------------------------------------------

def _q_blocks(role, n_blocks):
    """q-block indices (each 512 tokens) for a core role."""
    if n_blocks == 4:
        return [0, 3] if role == 0 else [1, 2]
    # degenerate small configs: one core covers all blocks
    return list(range(n_blocks))


def make_host_data(x, w_in, w_out, q_gamma, q_beta, k_gamma, k_beta, cfg,
                   n_cores=None):
    """Build per-core in_maps (list of dicts) + assembly metadata."""
    D = cfg["D"]
    S = cfg["S"]
    NQTOK = cfg["NQTOK"]
    QT = cfg["QT"]
    slots = cfg["slots"]
    masked = cfg["masked"]
    NH = D // HD
    NQ = NQTOK // QT
    MAXM = max(len(m) for m in masked)
    if cfg.get("MASK_F32", True):
        mdt = np.float32
    else:
        import ml_dtypes
        mdt = ml_dtypes.bfloat16
    B = x.shape[0]
    n_blocks = S // 512
    if n_cores is None:
        n_cores = B * (2048 // NQTOK) if S == 2048 else B

    w64 = np.asarray(w_in, np.float64)
    wq = w64[0:D]
    wk = w64[D:2 * D]
    wv = w64[2 * D:3 * D]
    wq_c = wq - wq.mean(axis=0, keepdims=True)
    wk_c = wk - wk.mean(axis=0, keepdims=True)
    wqkT2 = np.concatenate([wq_c.T, wk_c.T], axis=1).astype(np.float32)
    # pre-tile to [2*NH, P, DC, P]: tile ec -> [p, dc, e] with contiguous rows
    NHl = D // P
    DCl = D // P
    wqkT = np.ascontiguousarray(
        wqkT2.reshape(DCl, P, 2 * NHl, P).transpose(2, 1, 0, 3)
    )
    wvT = np.ascontiguousarray(wv.T.astype(np.float32))
    woT = np.ascontiguousarray(np.asarray(w_out, np.float64).T.astype(np.float32))

    inv = 1.0 / (10000.0 ** (np.arange(0, HD, 2, dtype=np.float64) / HD))
    tpos = np.arange(S, dtype=np.float64)
    fr = np.outer(tpos, inv)
    emb = np.concatenate([fr, fr], axis=-1)  # [S, HD]
    cosT = np.cos(emb).T  # [HD, S]
    sinTn = np.sin(emb).T

    # signed rotate-half permutation, as matmul lhsT:
    # out[p] = sum_{p'} rotmT[p', p] * in[p'] = rot_half(in)[p]
    h2 = HD // 2
    rotmT = np.zeros((P, P), np.float32)
    for p in range(h2):
        rotmT[p + h2, p] = -1.0
    for p in range(h2, HD):
        rotmT[p - h2, p] = 1.0

    scale = 1.0 / math.sqrt(HD)
    gq_a = np.ascontiguousarray(
        (np.asarray(q_gamma, np.float64) * scale).reshape(NH, P).T
    ).astype(np.float32)
    bq_a = np.ascontiguousarray(
        (np.asarray(q_beta, np.float64) * scale).reshape(NH, P).T
    ).astype(np.float32)
    gk_a = np.ascontiguousarray(
        np.asarray(k_gamma, np.float32).reshape(NH, P).T
    )
    bk_a = np.ascontiguousarray(
        np.asarray(k_beta, np.float32).reshape(NH, P).T
    )

    in_maps = []
    meta = []
    cores_per_batch = max(1, n_cores // B)
    for c in range(n_cores):
        b = c // cores_per_batch
        r = c % cores_per_batch
        blocks = _q_blocks(r if cores_per_batch > 1 else 0, n_blocks)
        blocks = blocks[: NQTOK // 512]
        qtok = np.concatenate(
            [np.arange(bk * 512, (bk + 1) * 512) for bk in blocks]
        )
        xb = np.asarray(x[b], np.float32)  # [S, D]
        xT = np.ascontiguousarray(xb.T)    # [D, S]
        xTq = np.ascontiguousarray(xT[:, qtok])
        cosq = np.ascontiguousarray(cosT[:, qtok].astype(np.float32))
        sinq = np.ascontiguousarray(sinTn[:, qtok].astype(np.float32))
        cosk = np.ascontiguousarray(cosT[:, :S].astype(np.float32))
        sink = np.ascontiguousarray(sinTn[:, :S].astype(np.float32))

        masks = np.zeros([NQ, P, MAXM, QT], np.float32)
        for t in range(NQ):
            q_start = blocks[t * (QT // 512)] * 512 if QT == 512 else None
            assert QT == 512
            q_start = blocks[t] * 512
            qq = np.arange(QT)
            kk = np.arange(P)
            for mi, kc in enumerate(masked[t]):
                masks[t, :, mi, :] = (
                    (kc * P + kk[:, None]) <= (q_start + qq[None, :])
                ).astype(np.float32)
        masks = masks.astype(mdt)

        in_maps.append(dict(
            xTq=xTq, xT=xT, wqkT=wqkT, wvT=wvT, woT=woT,
            cosq=cosq, sinqn=sinq, cosk=cosk, sinkn=sink,
            gq=gq_a, bq=bq_a, gk=gk_a, bk=bk_a, masks=masks,
            rotm=rotmT,
            onesc=np.ones((P, 1), np.float32),
            onesr=np.ones((1, P), np.float32),
        ))
        meta.append(dict(b=b, qtok=qtok))
    return in_maps, meta


_PROGRAM_CACHE = {}


def _get_program(cfg_key, cfg):
    if cfg_key not in _PROGRAM_CACHE:
        _PROGRAM_CACHE[cfg_key] = build_program(cfg)
    return _PROGRAM_CACHE[cfg_key]


def run_full(x, w_in, w_out, q_gamma, q_beta, k_gamma, k_beta,
             trace=False):
    from concourse.bass_utils import run_bass_kernel_spmd

    cfg = FULL_CFG
    B = x.shape[0]
    n_cores = 2 * B
    in_maps, meta = make_host_data(
        x, w_in, w_out, q_gamma, q_beta, k_gamma, k_beta, cfg,
        n_cores=n_cores,
    )
    nc = _get_program("full", cfg)
    res = run_bass_kernel_spmd(
        nc, in_maps, core_ids=list(range(n_cores)), trace=trace,
    )
    S, D = cfg["S"], cfg["D"]
    out = np.empty((B, S, D), np.float32)
    for c in range(n_cores):
        o = res.results[c]["out"]  # [D, NQTOK]
        out[meta[c]["b"], meta[c]["qtok"], :] = o.T
    return out, res


def kernel(x, w_in, w_out, q_gamma, q_beta, k_gamma, k_beta, n_heads=16,
           **_ignored):
    x = np.asarray(x, np.float32)
    assert int(np.asarray(n_heads)) * HD == x.shape[-1]
    out, _ = run_full(
        np.asarray(x, np.float32),
        np.asarray(w_in, np.float32),
        np.asarray(w_out, np.float32),
        np.asarray(q_gamma, np.float32),
        np.asarray(q_beta, np.float32),
        np.asarray(k_gamma, np.float32),
        np.asarray(k_beta, np.float32),
    )
    return out



# revision 6
# speedup vs baseline: 1.2516x; 1.2516x over previous
"""Trainium2 Bass kernel for a custom attention block (qkv-proj + LN(q,k) +
RoPE + causal attention + out-proj), distributed over 8 NeuronCores.

Sharding: 2 cores per batch (B=4). Core role r=c%2 takes q-token blocks
{0,3} (r=0) or {1,2} (r=1) of 512 tokens; every core computes K/V for the
full 2048-token sequence of its batch (no collectives). The compiled
program is identical on all cores; per-core differences are input data
only. To keep the q-slab offsets compile-time-constant, each core sees
the sequence in a per-role BLOCK PERMUTATION (r=0: 0,1,2,3; r=1:
1,0,3,2), so its q blocks always sit at permuted positions {0,3}. The
cos/sin tables, causal masks and output assembly are permutation-aware
host data.

All matmuls run in bf16 (same PE rate as fp32r but faster weight loads
via fast-weight-load, half the DMA/SBUF), with fp32 PSUM accumulation.
x is SBUF-resident; q stays SBUF-resident post-rope; k/v round-trip
through DRAM scratch.

LN: mean subtraction is folded into host-pre-centered w_in rows;
variance comes from Square + ones-matmul partition reduction;
rsqrt(var+eps) = Exp(-0.5*Ln(var+eps)).
"""

import math

import numpy as np

import concourse.bass as bass
import concourse.mybir as mybir
import concourse.tile as tile
from concourse import bacc
from concourse.bass import ds

F32 = mybir.dt.float32
F32R = mybir.dt.float32r
BF16 = mybir.dt.bfloat16
AF = mybir.ActivationFunctionType
OP = mybir.AluOpType

P = 128
HD = 128
D = 2048
S = 2048
NH = D // HD          # 16 heads = feature chunks
DC = D // P           # 16 contraction chunks
NQTOK = 1024          # q tokens per core
QT = 512              # q/attention tile width (moving dim)
NQ = NQTOK // QT      # 2 q tiles per core
EXP_BIAS = 8.0
EPS = 1e-5
SLOTS = (8, 16)       # kv 128-chunks per q tile (max over the two roles)
MASKED = (tuple(range(0, 8)), tuple(range(8, 16)))
MAXM = 8
Q_POS = (0, 3)        # structural (permuted) block positions of q slabs
VET = 4               # v feature tiles of 512
VEW = 512
LOOKAHEAD = 2         # attention score-slot software pipeline depth


def _r(ap):
    """fp32 -> fp32r view for matmul operands."""
    return ap.bitcast(F32R)


def build_program():
    nc = bacc.Bacc("TRN2", target_bir_lowering=False, debug=False)

    # ---- I/O ----
    xT_i = nc.dram_tensor("xT", [D, S], BF16, kind="ExternalInput").ap()
    wqk_i = nc.dram_tensor("wqk", [2 * NH, P, DC, P], BF16,
                           kind="ExternalInput").ap()
    wv_i = nc.dram_tensor("wv", [VET, P, DC, VEW], BF16,
                          kind="ExternalInput").ap()
    wo_i = nc.dram_tensor("wo", [NH, P, NH, P], BF16,
                          kind="ExternalInput").ap()
    cos_i = nc.dram_tensor("cos", [HD, S], BF16, kind="ExternalInput").ap()
    sin_i = nc.dram_tensor("sin", [HD, S], BF16, kind="ExternalInput").ap()
    gq_i = nc.dram_tensor("gq", [P, NH], F32, kind="ExternalInput").ap()
    bq_i = nc.dram_tensor("bq", [P, NH], F32, kind="ExternalInput").ap()
    gk_i = nc.dram_tensor("gk", [P, NH], F32, kind="ExternalInput").ap()
    bk_i = nc.dram_tensor("bk", [P, NH], F32, kind="ExternalInput").ap()
    masks_i = nc.dram_tensor("masks", [NQ, P, MAXM, QT], BF16,
                             kind="ExternalInput").ap()
    onesc_i = nc.dram_tensor("onesc", [P, 1], BF16, kind="ExternalInput").ap()
    onesr_i = nc.dram_tensor("onesr", [1, P], F32, kind="ExternalInput").ap()
    rotm_i = nc.dram_tensor("rotm", [P, P], BF16, kind="ExternalInput").ap()
    out_t = nc.dram_tensor("out", [D, NQTOK], F32, kind="ExternalOutput").ap()

    with tile.TileContext(nc) as tc:
        import contextlib

        ctx = contextlib.ExitStack()
        with ctx:
            sb = ctx.enter_context(tc.tile_pool(name="sb", bufs=1))
            psum = ctx.enter_context(tc.tile_pool(name="ps", bufs=1, space="PSUM"))
            dram = ctx.enter_context(tc.tile_pool(name="dram", bufs=1, space="DRAM"))

            # ---- DRAM scratch ----
            kts = dram.tile([P, NH, S], BF16, tag="kts", name="kts")
            vs = dram.tile([NH, S, HD], BF16, tag="vs", name="vs")

            # ---- constants / small inputs ----
            ones_col = sb.tile([P, 1], BF16, tag="ones_col", name="ones_col")
            nc.sync.dma_start(ones_col, onesc_i)
            ones_row = sb.tile([1, P], F32, tag="ones_row", name="ones_row")
            nc.sync.dma_start(_r(ones_row), _r(onesr_i))
            eps1 = sb.tile([1, 1], F32, tag="eps1", name="eps1")
            nc.vector.memset(eps1, EPS)
            zero1 = sb.tile([1, 1], F32, tag="zero1", name="zero1")
            nc.vector.memset(zero1, 0.0)
            nege = sb.tile([P, 1], F32, tag="nege", name="nege")
            nc.vector.memset(nege, -EXP_BIAS)
            rotm = sb.tile([P, P], BF16, tag="rotm", name="rotm")
            nc.sync.dma_start(rotm, rotm_i)
            gq = sb.tile([P, NH], F32, tag="gq", name="gq")
            nc.sync.dma_start(gq, gq_i)
            bq = sb.tile([P, NH], F32, tag="bq", name="bq")
            nc.sync.dma_start(bq, bq_i)
            gk = sb.tile([P, NH], F32, tag="gk", name="gk")
            nc.sync.dma_start(gk, gk_i)
            bk = sb.tile([P, NH], F32, tag="bk", name="bk")
            nc.sync.dma_start(bk, bk_i)
            cos_t = sb.tile([HD, S], BF16, tag="cos_t", name="cos_t")
            nc.sync.dma_start(cos_t, cos_i)
            sin_t = sb.tile([HD, S], BF16, tag="sin_t", name="sin_t")
            nc.sync.dma_start(sin_t, sin_i)

            # ---- resident x: [128, DC, S] bf16 (64KB/partition) ----
            xsb = sb.tile([P, DC, S], BF16, tag="xsb", bufs=1, name="xsb")
            for d in range(DC):
                nc.sync.dma_start(xsb[:, d], xT_i[ds(d * P, P), :])

            # ---- resident q (post-LN+rope): [128, NH, NQTOK] bf16 ----
            q_res = sb.tile([P, NH, NQTOK], BF16, tag="q_res", bufs=1,
                            name="q_res")

            def proj_group(ec_base, slab_offs, q_dst_offs, g_sb, b_sb):
                """Project x -> feature-partition [128, QT] tiles for each
                token slab in this group (2 slabs), with LN + rope.

                slab_offs: compile-time token offsets into the permuted
                sequence (index into x and cos/sin). If q_dst_offs is not
                None the result lands at q_res[:, :, q_dst_off]; else it
                DMAs to kts[:, :, slab_off].
                """
                n_s = len(slab_offs)
                holds = []
                for i in range(n_s):
                    if q_dst_offs is not None:
                        holds.append(q_res[:, :, ds(q_dst_offs[i], QT)])
                    else:
                        h = sb.tile([P, NH, QT], BF16, tag="khold", bufs=2,
                                    name="khold")
                        holds.append(h)
                pstats = []
                for i in range(n_s):
                    pstats.append(psum.tile([1, QT], F32, tag="stat", bufs=3,
                                            name="ps_stat"))
                for ec in range(NH):
                    w = sb.tile([P, DC, P], BF16, tag="w", bufs=2, name="w")
                    nc.sync.dma_start(w, wqk_i[ec_base + ec])
                    pss = [psum.tile([P, QT], F32, tag="mm", bufs=5, name="ps")
                           for _ in range(n_s)]
                    for d in range(DC):
                        for i in range(n_s):
                            nc.tensor.matmul(
                                pss[i],
                                lhsT=w[:, d],
                                rhs=xsb[:, d, ds(slab_offs[i], QT)],
                                start=(d == 0),
                                stop=(d == DC - 1),
                            )
                    for i in range(n_s):
                        nc.vector.tensor_copy(holds[i][:, ec], pss[i])
                        sq = sb.tile([P, QT], BF16, tag="sq", bufs=2,
                                     name="sq")
                        nc.scalar.square(sq, pss[i])
                        nc.tensor.matmul(
                            pstats[i],
                            lhsT=ones_col,
                            rhs=sq,
                            start=(ec == 0),
                            stop=(ec == NH - 1),
                        )
                for i in range(n_s):
                    hold = holds[i]
                    csl = ds(slab_offs[i], QT)
                    # rsig = exp(-0.5 * ln(sumsq/D + eps))
                    lnv = sb.tile([1, QT], F32, tag="stats_sb", bufs=4,
                                  name="lnv")
                    nc.scalar.activation(lnv, pstats[i], AF.Ln,
                                         scale=1.0 / D, bias=eps1)
                    rsig = sb.tile([1, QT], F32, tag="stats_sb", bufs=4,
                                   name="rsig")
                    nc.scalar.activation(_r(rsig), lnv, AF.Exp, bias=zero1,
                                         scale=-0.5)
                    ps_rep = psum.tile([P, QT], F32, tag="mm", bufs=5,
                                       name="ps_rep")
                    nc.tensor.matmul(ps_rep, lhsT=_r(ones_row), rhs=_r(rsig))
                    # pass 1: LN apply on all chunks (DVE)
                    for ec in range(NH):
                        ch = hold[:, ec]
                        nc.vector.tensor_tensor(ch, ch, ps_rep, op=OP.mult)
                        nc.vector.tensor_scalar(
                            ch, ch,
                            scalar1=g_sb[:, ds(ec, 1)],
                            scalar2=b_sb[:, ds(ec, 1)],
                            op0=OP.mult, op1=OP.add,
                        )
                    # pass 2: rope rotation matmuls stream back-to-back
                    for ec in range(NH):
                        ch = hold[:, ec]
                        ps_rot = psum.tile([P, QT], F32, tag="mm", bufs=5,
                                           name="ps_rot")
                        nc.tensor.matmul(ps_rot, lhsT=rotm, rhs=ch)
                        tmp = sb.tile([P, QT], BF16, tag="rtmp", bufs=2,
                                      name="rtmp")
                        nc.vector.tensor_tensor(tmp, ps_rot, sin_t[:, csl],
                                                op=OP.mult)
                        nc.vector.tensor_tensor(ch, ch, cos_t[:, csl],
                                                op=OP.mult)
                        nc.vector.tensor_tensor(ch, ch, tmp, op=OP.add)
                    if q_dst_offs is None:
                        nc.sync.dma_start(
                            kts[:, :, ds(slab_offs[i], QT)], hold
                        )

            # ---- Phase Q: q projection (permuted positions 0 and 3) ----
            proj_group(0, [Q_POS[0] * QT, Q_POS[1] * QT], [0, QT], gq, bq)

            # ---- Phase K: 2 groups of 2 slabs ----
            proj_group(NH, [0, QT], None, gk, bk)
            proj_group(NH, [2 * QT, 3 * QT], None, gk, bk)

            # ---- Phase V ----
            KC = S // P  # 16 kv chunks
            for et in range(VET):
                for scg in range(KC // 2):
                    scs = [scg * 2, scg * 2 + 1]
                    psv = {sc: psum.tile([P, VEW], F32, tag="mm", bufs=5,
                                         name="psv")
                           for sc in scs}
                    for d in range(DC):
                        wvt = sb.tile([P, VEW], BF16, tag="wvt", bufs=4,
                                      name="wvt")
                        nc.sync.dma_start(wvt, wv_i[et, :, d])
                        for sc in scs:
                            nc.tensor.matmul(
                                psv[sc],
                                lhsT=xsb[:, d, ds(sc * P, P)],
                                rhs=wvt,
                                start=(d == 0),
                                stop=(d == DC - 1),
                            )
                    for sc in scs:
                        vsb = sb.tile([P, VEW], BF16, tag="vsb", bufs=3,
                                      name="vsb")
                        nc.vector.tensor_copy(vsb, psv[sc])
                        for hh in range(VEW // HD):
                            nc.sync.dma_start(
                                vs[et * (VEW // HD) + hh, ds(sc * P, P), :],
                                vsb[:, ds(hh * HD, HD)],
                            )

            # ---- Attention + out-projection per q tile ----
            for t in range(NQ):
                qsl_off = t * QT
                n_slots = SLOTS[t]
                mt = sb.tile([P, MAXM, QT], BF16, tag="masks", bufs=1,
                             name="mt")
                nc.sync.dma_start(mt, masks_i[t])
                mpos = {kc: i for i, kc in enumerate(MASKED[t])}
                ot_res = sb.tile([P, NH, QT], BF16, tag="khold", bufs=2,
                                 name="ot_res")
                for h in range(NH):
                    ksl = sb.tile([P, S], BF16, tag="kslab", bufs=2,
                                  name="ksl")
                    nc.sync.dma_start(ksl[:, ds(0, n_slots * P)],
                                      kts[:, h, ds(0, n_slots * P)])
                    vsl = sb.tile([P, KC, HD], BF16, tag="vslab", bufs=2,
                                  name="vsl")
                    nc.sync.dma_start(
                        vsl[:, ds(0, n_slots)],
                        vs[h, ds(0, n_slots * P), :].rearrange(
                            "(kc p) hd -> p kc hd", p=P
                        ),
                    )
                    psout = psum.tile([P, QT], F32, tag="mm", bufs=5,
                                      name="psout")
                    psden = psum.tile([1, QT], F32, tag="stat", bufs=3,
                                      name="psden")
                    qsl = q_res[:, h, ds(qsl_off, QT)]

                    ets = {}

                    def emit_score(s):
                        pss = psum.tile([P, QT], F32, tag="mm", bufs=5,
                                        name="pss")
                        nc.tensor.matmul(pss, lhsT=ksl[:, ds(s * P, P)],
                                         rhs=qsl)
                        et = sb.tile([P, QT], BF16, tag="exp", bufs=4,
                                     name="et")
                        nc.scalar.activation(et, pss, AF.Exp, bias=nege)
                        if s in mpos:
                            nc.vector.tensor_tensor(
                                et, et, mt[:, mpos[s]], op=OP.mult
                            )
                        ets[s] = et

                    for s in range(min(LOOKAHEAD, n_slots)):
                        emit_score(s)
                    for s in range(n_slots):
                        if s + LOOKAHEAD < n_slots:
                            emit_score(s + LOOKAHEAD)
                        et = ets.pop(s)
                        nc.tensor.matmul(
                            psout,
                            lhsT=vsl[:, s],
                            rhs=et,
                            start=(s == 0),
                            stop=(s == n_slots - 1),
                        )
                        nc.tensor.matmul(
                            psden,
                            lhsT=ones_col,
                            rhs=et,
                            start=(s == 0),
                            stop=(s == n_slots - 1),
                        )
                    rec0 = sb.tile([1, QT], F32, tag="stats_sb", bufs=4,
                                   name="rec0")
                    with nc.allow_low_precision(
                        reason="denominator reciprocal, 18 bits is plenty"
                    ):
                        nc.vector.reciprocal_approx_fast(rec0, psden)
                    rec = sb.tile([1, QT], F32, tag="stats_sb", bufs=4,
                                  name="rec")
                    nc.scalar.activation(_r(rec), rec0, AF.Copy)
                    psr = psum.tile([P, QT], F32, tag="mm", bufs=5,
                                    name="psr")
                    nc.tensor.matmul(psr, lhsT=_r(ones_row), rhs=_r(rec))
                    nc.vector.tensor_copy(ot_res[:, h], psout)
                    nc.vector.tensor_tensor(ot_res[:, h], ot_res[:, h], psr,
                                            op=OP.mult)

                # ---- out-projection for this q tile ----
                for e in range(NH):
                    wot = sb.tile([P, NH, P], BF16, tag="wot", bufs=2,
                                  name="wot")
                    nc.sync.dma_start(wot, wo_i[e])
                    psf = psum.tile([P, QT], F32, tag="mm", bufs=5,
                                    name="psf")
                    for h in range(NH):
                        nc.tensor.matmul(
                            psf,
                            lhsT=wot[:, h],
                            rhs=ot_res[:, h],
                            start=(h == 0),
                            stop=(h == NH - 1),
                        )
                    fsb = sb.tile([P, QT], F32, tag="fsb", bufs=2,
                                  name="fsb")
                    nc.vector.tensor_copy(fsb, psf)
                    nc.sync.dma_start(
                        out_t[ds(e * P, P), ds(qsl_off, QT)], fsb
                    )

    nc.compile()
    return nc


# --------------------------------------------------------------------------
# Host-side prep and driver
# --------------------------------------------------------------------------

_PERMS = {0: (0, 1, 2, 3), 1: (1, 0, 3, 2)}


def make_host_data(x, w_in, w_out, q_gamma, q_beta, k_gamma, k_beta):
    """Build per-core in_maps (list of dicts) + assembly metadata."""
    import ml_dtypes
    bf16 = ml_dtypes.bfloat16

    B = x.shape[0]
    n_cores = 2 * B

    w64 = np.asarray(w_in, np.float64)
    wq = w64[0:D]
    wk = w64[D:2 * D]
    wv = w64[2 * D:3 * D]
    wq_c = wq - wq.mean(axis=0, keepdims=True)
    wk_c = wk - wk.mean(axis=0, keepdims=True)
    wqkT2 = np.concatenate([wq_c.T, wk_c.T], axis=1)   # [D, 2D]
    wqk_t = np.ascontiguousarray(
        wqkT2.reshape(DC, P, 2 * NH, P).transpose(2, 1, 0, 3)
    ).astype(bf16)
    wvT = wv.T  # [D, D]
    wv_t = np.ascontiguousarray(
        wvT.reshape(DC, P, VET, VEW).transpose(2, 1, 0, 3)
    ).astype(bf16)
    woT = np.asarray(w_out, np.float64).T  # [D(hfeat), D(eout)]
    wo_t = np.ascontiguousarray(
        woT.reshape(NH, P, NH, P).transpose(2, 1, 0, 3)
    ).astype(bf16)

    inv = 1.0 / (10000.0 ** (np.arange(0, HD, 2, dtype=np.float64) / HD))
    tpos = np.arange(S, dtype=np.float64)
    fr = np.outer(tpos, inv)
    emb = np.concatenate([fr, fr], axis=-1)  # [S, HD]
    cosT = np.cos(emb).T  # [HD, S]
    sinT = np.sin(emb).T

    h2 = HD // 2
    rotmT = np.zeros((P, P), np.float32)
    for p in range(h2):
        rotmT[p + h2, p] = -1.0
    for p in range(h2, HD):
        rotmT[p - h2, p] = 1.0
    rotm = rotmT.astype(bf16)

    scale = 1.0 / math.sqrt(HD)
    gq_a = np.ascontiguousarray(
        (np.asarray(q_gamma, np.float64) * scale).reshape(NH, P).T
    ).astype(np.float32)
    bq_a = np.ascontiguousarray(
        (np.asarray(q_beta, np.float64) * scale).reshape(NH, P).T
    ).astype(np.float32)
    gk_a = np.ascontiguousarray(
        np.asarray(k_gamma, np.float32).reshape(NH, P).T
    )
    bk_a = np.ascontiguousarray(
        np.asarray(k_beta, np.float32).reshape(NH, P).T
    )
    onesc = np.ones((P, 1), bf16)
    onesr = np.ones((1, P), np.float32)

    xb_T = {}
    in_maps = []
    meta = []
    for c in range(n_cores):
        b = c // 2
        r = c % 2
        perm = _PERMS[r]
        ptok = np.concatenate(
            [np.arange(pb * QT, (pb + 1) * QT) for pb in perm]
        )
        if b not in xb_T:
            xb_T[b] = np.ascontiguousarray(
                np.asarray(x[b], np.float32).T
            )  # [D, S] f32
        xT = np.ascontiguousarray(xb_T[b][:, ptok]).astype(bf16)
        cosp = np.ascontiguousarray(cosT[:, ptok]).astype(bf16)
        sinp = np.ascontiguousarray(sinT[:, ptok]).astype(bf16)

        # masks in PERMUTED kv space; q slabs at permuted positions Q_POS
        masks = np.zeros([NQ, P, MAXM, QT], np.float32)
        for t in range(NQ):
            gq_tok = ptok[Q_POS[t] * QT + np.arange(QT)]
            for mi, kc in enumerate(MASKED[t]):
                gkv = ptok[kc * P + np.arange(P)]
                masks[t, :, mi, :] = (
                    gkv[:, None] <= gq_tok[None, :]
                ).astype(np.float32)
        masks = masks.astype(bf16)

        qtok = np.concatenate(
            [np.arange(perm[pq] * QT, (perm[pq] + 1) * QT) for pq in Q_POS]
        )
        in_maps.append(dict(
            xT=xT, wqk=wqk_t, wv=wv_t, wo=wo_t,
            cos=cosp, sin=sinp,
            gq=gq_a, bq=bq_a, gk=gk_a, bk=bk_a, masks=masks,
            onesc=onesc, onesr=onesr, rotm=rotm,
        ))
        meta.append(dict(b=b, qtok=qtok))
    return in_maps, meta


_PROGRAM_CACHE = {}


def _get_program():
    if "full" not in _PROGRAM_CACHE:
        _PROGRAM_CACHE["full"] = build_program()
    return _PROGRAM_CACHE["full"]


def run_full(x, w_in, w_out, q_gamma, q_beta, k_gamma, k_beta,
             trace=False):
    from concourse.bass_utils import run_bass_kernel_spmd

    B = x.shape[0]
    n_cores = 2 * B
    in_maps, meta = make_host_data(
        x, w_in, w_out, q_gamma, q_beta, k_gamma, k_beta,
    )
    nc = _get_program()
    res = run_bass_kernel_spmd(
        nc, in_maps, core_ids=list(range(n_cores)), trace=trace,
    )
    out = np.empty((B, S, D), np.float32)
    for c in range(n_cores):
        o = res.results[c]["out"]  # [D, NQTOK]
        out[meta[c]["b"], meta[c]["qtok"], :] = o.T
    return out, res


def kernel(x, w_in, w_out, q_gamma, q_beta, k_gamma, k_beta, n_heads=16,
           **_ignored):
    x = np.asarray(x, np.float32)
    assert int(np.asarray(n_heads)) * HD == x.shape[-1]
    out, _ = run_full(
        np.asarray(x, np.float32),
        np.asarray(w_in, np.float32),
        np.asarray(w_out, np.float32),
        np.asarray(q_gamma, np.float32),
        np.asarray(q_beta, np.float32),
        np.asarray(k_gamma, np.float32),
        np.asarray(k_beta, np.float32),
    )
    return out


# revision 16
# speedup vs baseline: 1.2683x; 1.0133x over previous
"""Trainium2 Bass kernel for a custom attention block (qkv-proj + LN(q,k) +
RoPE + causal attention + out-proj), distributed over 8 NeuronCores.

Sharding: 2 cores per batch (B=4). Core role r=c%2 takes q-token blocks
{0,3} (r=0) or {1,2} (r=1) of 512 tokens; every core computes K/V for the
full 2048-token sequence of its batch (no collectives). The compiled
program is identical on all cores; per-core differences are input data
only. To keep the q-slab offsets compile-time-constant, each core sees
the sequence in a per-role BLOCK PERMUTATION (r=0: 0,1,2,3; r=1:
1,0,3,2), so its q blocks always sit at permuted positions {0,3}. The
cos/sin tables, causal masks and output assembly are permutation-aware
host data.

All matmuls run in bf16 (same PE rate as fp32r but faster weight loads
via fast-weight-load, half the DMA/SBUF), with fp32 PSUM accumulation.
x is SBUF-resident; q stays SBUF-resident post-rope; k/v round-trip
through DRAM scratch.

LN: mean subtraction is folded into host-pre-centered w_in rows;
variance comes from Square + ones-matmul partition reduction;
rsqrt(var+eps) = Exp(-0.5*Ln(var+eps)).
"""

import math

import numpy as np

import concourse.bass as bass
import concourse.mybir as mybir
import concourse.tile as tile
from concourse import bacc
from concourse.bass import ds

F32 = mybir.dt.float32
F32R = mybir.dt.float32r
BF16 = mybir.dt.bfloat16
AF = mybir.ActivationFunctionType
OP = mybir.AluOpType

P = 128
HD = 128
D = 2048
S = 2048
NH = D // HD          # 16 heads = feature chunks
DC = D // P           # 16 contraction chunks
NQTOK = 1024          # q tokens per core
QT = 512              # q/attention tile width (moving dim)
NQ = NQTOK // QT      # 2 q tiles per core
EXP_BIAS = 8.0
EPS = 1e-5
SLOTS = (8, 16)       # kv 128-chunks per q tile (max over the two roles)
MASKED = (tuple(range(0, 8)), tuple(range(8, 16)))
MAXM = 8
Q_POS = (0, 3)        # structural (permuted) block positions of q slabs
VET = 4               # v feature tiles of 512
VEW = 512
LOOKAHEAD = 2         # attention score-slot software pipeline depth


def _r(ap):
    """fp32 -> fp32r view for matmul operands."""
    return ap.bitcast(F32R)


def build_program():
    nc = bacc.Bacc("TRN2", target_bir_lowering=False, debug=False)

    # ---- I/O ----
    xT_i = nc.dram_tensor("xT", [D, S], BF16, kind="ExternalInput").ap()
    wqk_i = nc.dram_tensor("wqk", [2 * NH, P, DC, P], BF16,
                           kind="ExternalInput").ap()
    wv_i = nc.dram_tensor("wv", [VET, P, DC, VEW], BF16,
                          kind="ExternalInput").ap()
    wo_i = nc.dram_tensor("wo", [NH, P, NH, P], BF16,
                          kind="ExternalInput").ap()
    cos_i = nc.dram_tensor("cos", [HD, S], BF16, kind="ExternalInput").ap()
    sin_i = nc.dram_tensor("sin", [HD, S], BF16, kind="ExternalInput").ap()
    gq_i = nc.dram_tensor("gq", [P, NH], F32, kind="ExternalInput").ap()
    bq_i = nc.dram_tensor("bq", [P, NH], F32, kind="ExternalInput").ap()
    gk_i = nc.dram_tensor("gk", [P, NH], F32, kind="ExternalInput").ap()
    bk_i = nc.dram_tensor("bk", [P, NH], F32, kind="ExternalInput").ap()
    masks_i = nc.dram_tensor("masks", [NQ, P, MAXM, QT], BF16,
                             kind="ExternalInput").ap()
    onesc_i = nc.dram_tensor("onesc", [P, 1], F32, kind="ExternalInput").ap()
    onesr_i = nc.dram_tensor("onesr", [1, P], F32, kind="ExternalInput").ap()
    rotm_i = nc.dram_tensor("rotm", [P, P], BF16, kind="ExternalInput").ap()
    out_t = nc.dram_tensor("out", [D, NQTOK], F32, kind="ExternalOutput").ap()

    with tile.TileContext(nc) as tc:
        import contextlib

        ctx = contextlib.ExitStack()
        with ctx:
            sb = ctx.enter_context(tc.tile_pool(name="sb", bufs=1))
            psum = ctx.enter_context(tc.tile_pool(name="ps", bufs=1, space="PSUM"))
            dram = ctx.enter_context(tc.tile_pool(name="dram", bufs=1, space="DRAM"))

            # ---- DRAM scratch ----
            kts = dram.tile([P, NH, S], BF16, tag="kts", name="kts")
            vs = dram.tile([VET, S, VEW], BF16, tag="vs", name="vs")

            # ---- constants / small inputs ----
            ones_col = sb.tile([P, 1], F32, tag="ones_col", name="ones_col")
            nc.sync.dma_start(_r(ones_col), _r(onesc_i))
            ones_row = sb.tile([1, P], F32, tag="ones_row", name="ones_row")
            nc.sync.dma_start(_r(ones_row), _r(onesr_i))
            eps1 = sb.tile([1, 1], F32, tag="eps1", name="eps1")
            nc.vector.memset(eps1, EPS)
            zero1 = sb.tile([1, 1], F32, tag="zero1", name="zero1")
            nc.vector.memset(zero1, 0.0)
            nege = sb.tile([P, 1], F32, tag="nege", name="nege")
            nc.vector.memset(nege, -EXP_BIAS)
            rotm = sb.tile([P, P], BF16, tag="rotm", name="rotm")
            nc.sync.dma_start(rotm, rotm_i)
            gq = sb.tile([P, NH], F32, tag="gq", name="gq")
            nc.sync.dma_start(gq, gq_i)
            bq = sb.tile([P, NH], F32, tag="bq", name="bq")
            nc.sync.dma_start(bq, bq_i)
            gk = sb.tile([P, NH], F32, tag="gk", name="gk")
            nc.sync.dma_start(gk, gk_i)
            bk = sb.tile([P, NH], F32, tag="bk", name="bk")
            nc.sync.dma_start(bk, bk_i)
            cos_t = sb.tile([HD, S], BF16, tag="cos_t", name="cos_t")
            nc.sync.dma_start(cos_t, cos_i)
            sin_t = sb.tile([HD, S], BF16, tag="sin_t", name="sin_t")
            nc.sync.dma_start(sin_t, sin_i)

            # ---- resident x: [128, DC, S] bf16 (64KB/partition) ----
            xsb = sb.tile([P, DC, S], BF16, tag="xsb", bufs=1, name="xsb")
            for d in range(DC):
                nc.sync.dma_start(xsb[:, d], xT_i[ds(d * P, P), :])

            # ---- resident q (post-LN+rope): [128, NH, NQTOK] bf16 ----
            q_res = sb.tile([P, NH, NQTOK], BF16, tag="q_res", bufs=1,
                            name="q_res")

            def proj_group(ec_base, slab_offs, q_dst_offs, g_sb, b_sb):
                """Project x -> feature-partition [128, QT] tiles for each
                token slab in this group (2 slabs), with LN + rope.

                slab_offs: compile-time token offsets into the permuted
                sequence (index into x and cos/sin). If q_dst_offs is not
                None the result lands at q_res[:, :, q_dst_off]; else it
                DMAs to kts[:, :, slab_off].
                """
                n_s = len(slab_offs)
                holds = []
                for i in range(n_s):
                    if q_dst_offs is not None:
                        holds.append(q_res[:, :, ds(q_dst_offs[i], QT)])
                    else:
                        h = sb.tile([P, NH, QT], BF16, tag="khold", bufs=2,
                                    name="khold")
                        holds.append(h)
                sqsums = []
                for i in range(n_s):
                    sqsums.append(sb.tile([P, QT], F32, tag="acc", bufs=2,
                                          name="sqsum"))
                for ec in range(NH):
                    w = sb.tile([P, DC, P], BF16, tag="w", bufs=2, name="w")
                    nc.sync.dma_start(w, wqk_i[ec_base + ec])
                    pss = [psum.tile([P, QT], F32, tag="mm", bufs=5, name="ps")
                           for _ in range(n_s)]
                    for d in range(DC):
                        for i in range(n_s):
                            nc.tensor.matmul(
                                pss[i],
                                lhsT=w[:, d],
                                rhs=xsb[:, d, ds(slab_offs[i], QT)],
                                start=(d == 0),
                                stop=(d == DC - 1),
                            )
                    for i in range(n_s):
                        nc.vector.tensor_copy(holds[i][:, ec], pss[i])
                        sq = sb.tile([P, QT], BF16, tag="sq", bufs=2,
                                     name="sq")
                        nc.scalar.square(sq, pss[i])
                        if ec == 0:
                            nc.vector.tensor_copy(_r(sqsums[i]), sq)
                        else:
                            nc.vector.tensor_tensor(_r(sqsums[i]), sqsums[i],
                                                    sq, op=OP.add)
                for i in range(n_s):
                    hold = holds[i]
                    csl = ds(slab_offs[i], QT)
                    # per-token sumsq: partition-sum of sqsum via ones-matmul
                    pstat = psum.tile([1, QT], F32, tag="stat", bufs=3,
                                      name="pstat")
                    nc.tensor.matmul(pstat, lhsT=_r(ones_col),
                                     rhs=_r(sqsums[i]))
                    # rsig = exp(-0.5 * ln(sumsq/D + eps))
                    lnv = sb.tile([1, QT], F32, tag="stats_sb", bufs=4,
                                  name="lnv")
                    nc.scalar.activation(lnv, pstat, AF.Ln,
                                         scale=1.0 / D, bias=eps1)
                    rsig = sb.tile([1, QT], F32, tag="stats_sb", bufs=4,
                                   name="rsig")
                    nc.scalar.activation(_r(rsig), lnv, AF.Exp, bias=zero1,
                                         scale=-0.5)
                    ps_rep = psum.tile([P, QT], F32, tag="mm", bufs=5,
                                       name="ps_rep")
                    nc.tensor.matmul(ps_rep, lhsT=_r(ones_row), rhs=_r(rsig))
                    # pass 1: LN apply on all chunks (DVE)
                    for ec in range(NH):
                        ch = hold[:, ec]
                        nc.vector.tensor_tensor(ch, ch, ps_rep, op=OP.mult)
                        nc.vector.tensor_scalar(
                            ch, ch,
                            scalar1=g_sb[:, ds(ec, 1)],
                            scalar2=b_sb[:, ds(ec, 1)],
                            op0=OP.mult, op1=OP.add,
                        )
                    # pass 2: rope rotation matmuls stream back-to-back
                    for ec in range(NH):
                        ch = hold[:, ec]
                        ps_rot = psum.tile([P, QT], F32, tag="mm", bufs=5,
                                           name="ps_rot")
                        nc.tensor.matmul(ps_rot, lhsT=rotm, rhs=ch)
                        tmp = sb.tile([P, QT], BF16, tag="rtmp", bufs=2,
                                      name="rtmp")
                        nc.vector.tensor_tensor(tmp, ps_rot, sin_t[:, csl],
                                                op=OP.mult)
                        nc.vector.tensor_tensor(ch, ch, cos_t[:, csl],
                                                op=OP.mult)
                        nc.vector.tensor_tensor(ch, ch, tmp, op=OP.add)
                    if q_dst_offs is None:
                        nc.sync.dma_start(
                            kts[:, :, ds(slab_offs[i], QT)], hold
                        )

            # ---- Phase Q: q projection (permuted positions 0 and 3) ----
            proj_group(0, [Q_POS[0] * QT, Q_POS[1] * QT], [0, QT], gq, bq)

            # ---- Phase K: 2 groups of 2 slabs ----
            proj_group(NH, [0, QT], None, gk, bk)
            proj_group(NH, [2 * QT, 3 * QT], None, gk, bk)

            # ---- Phase V ----
            KC = S // P  # 16 kv chunks
            for et in range(VET):
                for scg in range(KC // 2):
                    scs = [scg * 2, scg * 2 + 1]
                    psv = {sc: psum.tile([P, VEW], F32, tag="mm", bufs=5,
                                         name="psv")
                           for sc in scs}
                    for d in range(DC):
                        wvt = sb.tile([P, VEW], BF16, tag="wvt", bufs=4,
                                      name="wvt")
                        nc.sync.dma_start(wvt, wv_i[et, :, d])
                        for sc in scs:
                            nc.tensor.matmul(
                                psv[sc],
                                lhsT=xsb[:, d, ds(sc * P, P)],
                                rhs=wvt,
                                start=(d == 0),
                                stop=(d == DC - 1),
                            )
                    for sc in scs:
                        vsb = sb.tile([P, VEW], BF16, tag="vsb", bufs=3,
                                      name="vsb")
                        nc.vector.tensor_copy(vsb, psv[sc])
                        nc.sync.dma_start(vs[et, ds(sc * P, P), :], vsb)

            # ---- Attention + out-projection per q tile ----
            for t in range(NQ):
                qsl_off = t * QT
                n_slots = SLOTS[t]
                # masks overlay the (now dead) cos/sin buffers
                mt_lo = sb.tile([P, MAXM // 2, QT], BF16, tag="cos_t",
                                bufs=1, name="mt_lo")
                nc.sync.dma_start(mt_lo, masks_i[t][:, ds(0, MAXM // 2)])
                mt_hi = sb.tile([P, MAXM // 2, QT], BF16, tag="sin_t",
                                bufs=1, name="mt_hi")
                nc.sync.dma_start(mt_hi, masks_i[t][:, ds(MAXM // 2, MAXM // 2)])
                mpos = {kc: i for i, kc in enumerate(MASKED[t])}
                ot_res = sb.tile([P, NH, QT], BF16, tag="khold", bufs=2,
                                 name="ot_res")
                for h in range(NH):
                    ksl = sb.tile([P, S], BF16, tag="kslab", bufs=2,
                                  name="ksl")
                    nc.sync.dma_start(ksl[:, ds(0, n_slots * P)],
                                      kts[:, h, ds(0, n_slots * P)])
                    vsl = sb.tile([P, KC, HD], BF16, tag="vslab", bufs=2,
                                  name="vsl")
                    nc.sync.dma_start(
                        vsl[:, ds(0, n_slots)],
                        vs[h // 4, ds(0, n_slots * P),
                           ds((h % 4) * HD, HD)].rearrange(
                            "(kc p) hd -> p kc hd", p=P
                        ),
                    )
                    psout = psum.tile([P, QT], F32, tag="mm", bufs=5,
                                      name="psout")
                    esum = sb.tile([P, QT], F32, tag="acc", bufs=2,
                                   name="esum")
                    qsl = q_res[:, h, ds(qsl_off, QT)]

                    ets = {}

                    def emit_score(s):
                        pss = psum.tile([P, QT], F32, tag="mm", bufs=5,
                                        name="pss")
                        nc.tensor.matmul(pss, lhsT=ksl[:, ds(s * P, P)],
                                         rhs=qsl)
                        et = sb.tile([P, QT], BF16, tag="exp", bufs=4,
                                     name="et")
                        nc.scalar.activation(et, pss, AF.Exp, bias=nege)
                        if s in mpos:
                            mi = mpos[s]
                            msl = (mt_lo[:, mi] if mi < MAXM // 2
                                   else mt_hi[:, mi - MAXM // 2])
                            nc.vector.tensor_tensor(et, et, msl, op=OP.mult)
                        ets[s] = et

                    for s in range(min(LOOKAHEAD, n_slots)):
                        emit_score(s)
                    for s in range(n_slots):
                        if s + LOOKAHEAD < n_slots:
                            emit_score(s + LOOKAHEAD)
                        et = ets.pop(s)
                        nc.tensor.matmul(
                            psout,
                            lhsT=vsl[:, s],
                            rhs=et,
                            start=(s == 0),
                            stop=(s == n_slots - 1),
                        )
                        # denominator accumulation off the PE
                        if s == 0:
                            nc.vector.tensor_copy(_r(esum), et)
                        else:
                            nc.vector.tensor_tensor(_r(esum), esum, et,
                                                    op=OP.add)
                    psden = psum.tile([1, QT], F32, tag="stat", bufs=3,
                                      name="psden")
                    nc.tensor.matmul(psden, lhsT=_r(ones_col), rhs=_r(esum))
                    rec0 = sb.tile([1, QT], F32, tag="stats_sb", bufs=4,
                                   name="rec0")
                    with nc.allow_low_precision(
                        reason="denominator reciprocal, 18 bits is plenty"
                    ):
                        nc.vector.reciprocal_approx_fast(rec0, psden)
                    rec = sb.tile([1, QT], F32, tag="stats_sb", bufs=4,
                                  name="rec")
                    nc.scalar.activation(_r(rec), rec0, AF.Copy)
                    psr = psum.tile([P, QT], F32, tag="mm", bufs=5,
                                    name="psr")
                    nc.tensor.matmul(psr, lhsT=_r(ones_row), rhs=_r(rec))
                    nc.vector.tensor_copy(ot_res[:, h], psout)
                    nc.vector.tensor_tensor(ot_res[:, h], ot_res[:, h], psr,
                                            op=OP.mult)

                # ---- out-projection for this q tile ----
                for e in range(NH):
                    wot = sb.tile([P, NH, P], BF16, tag="wot", bufs=2,
                                  name="wot")
                    nc.sync.dma_start(wot, wo_i[e])
                    psf = psum.tile([P, QT], F32, tag="mm", bufs=5,
                                    name="psf")
                    for h in range(NH):
                        nc.tensor.matmul(
                            psf,
                            lhsT=wot[:, h],
                            rhs=ot_res[:, h],
                            start=(h == 0),
                            stop=(h == NH - 1),
                        )
                    fsb = sb.tile([P, QT], F32, tag="fsb", bufs=2,
                                  name="fsb")
                    nc.vector.tensor_copy(fsb, psf)
                    nc.sync.dma_start(
                        out_t[ds(e * P, P), ds(qsl_off, QT)], fsb
                    )

    nc.compile()
    return nc


# --------------------------------------------------------------------------
# Host-side prep and driver
# --------------------------------------------------------------------------

_PERMS = {0: (0, 1, 2, 3), 1: (1, 0, 3, 2)}


def make_host_data(x, w_in, w_out, q_gamma, q_beta, k_gamma, k_beta):
    """Build per-core in_maps (list of dicts) + assembly metadata."""
    import ml_dtypes
    bf16 = ml_dtypes.bfloat16

    B = x.shape[0]
    n_cores = 2 * B

    w64 = np.asarray(w_in, np.float64)
    wq = w64[0:D]
    wk = w64[D:2 * D]
    wv = w64[2 * D:3 * D]
    wq_c = wq - wq.mean(axis=0, keepdims=True)
    wk_c = wk - wk.mean(axis=0, keepdims=True)
    wqkT2 = np.concatenate([wq_c.T, wk_c.T], axis=1)   # [D, 2D]
    wqk_t = np.ascontiguousarray(
        wqkT2.reshape(DC, P, 2 * NH, P).transpose(2, 1, 0, 3)
    ).astype(bf16)
    wvT = wv.T  # [D, D]
    wv_t = np.ascontiguousarray(
        wvT.reshape(DC, P, VET, VEW).transpose(2, 1, 0, 3)
    ).astype(bf16)
    woT = np.asarray(w_out, np.float64).T  # [D(hfeat), D(eout)]
    wo_t = np.ascontiguousarray(
        woT.reshape(NH, P, NH, P).transpose(2, 1, 0, 3)
    ).astype(bf16)

    inv = 1.0 / (10000.0 ** (np.arange(0, HD, 2, dtype=np.float64) / HD))
    tpos = np.arange(S, dtype=np.float64)
    fr = np.outer(tpos, inv)
    emb = np.concatenate([fr, fr], axis=-1)  # [S, HD]
    cosT = np.cos(emb).T  # [HD, S]
    sinT = np.sin(emb).T

    h2 = HD // 2
    rotmT = np.zeros((P, P), np.float32)
    for p in range(h2):
        rotmT[p + h2, p] = -1.0
    for p in range(h2, HD):
        rotmT[p - h2, p] = 1.0
    rotm = rotmT.astype(bf16)

    scale = 1.0 / math.sqrt(HD)
    gq_a = np.ascontiguousarray(
        (np.asarray(q_gamma, np.float64) * scale).reshape(NH, P).T
    ).astype(np.float32)
    bq_a = np.ascontiguousarray(
        (np.asarray(q_beta, np.float64) * scale).reshape(NH, P).T
    ).astype(np.float32)
    gk_a = np.ascontiguousarray(
        np.asarray(k_gamma, np.float32).reshape(NH, P).T
    )
    bk_a = np.ascontiguousarray(
        np.asarray(k_beta, np.float32).reshape(NH, P).T
    )
    onesc = np.ones((P, 1), np.float32)
    onesr = np.ones((1, P), np.float32)

    xb_T = {}
    in_maps = []
    meta = []
    for c in range(n_cores):
        b = c // 2
        r = c % 2
        perm = _PERMS[r]
        ptok = np.concatenate(
            [np.arange(pb * QT, (pb + 1) * QT) for pb in perm]
        )
        if b not in xb_T:
            xb_T[b] = np.ascontiguousarray(
                np.asarray(x[b], np.float32).T
            )  # [D, S] f32
        xT = np.ascontiguousarray(xb_T[b][:, ptok]).astype(bf16)
        cosp = np.ascontiguousarray(cosT[:, ptok]).astype(bf16)
        sinp = np.ascontiguousarray(sinT[:, ptok]).astype(bf16)

        # masks in PERMUTED kv space; q slabs at permuted positions Q_POS
        masks = np.zeros([NQ, P, MAXM, QT], np.float32)
        for t in range(NQ):
            gq_tok = ptok[Q_POS[t] * QT + np.arange(QT)]
            for mi, kc in enumerate(MASKED[t]):
                gkv = ptok[kc * P + np.arange(P)]
                masks[t, :, mi, :] = (
                    gkv[:, None] <= gq_tok[None, :]
                ).astype(np.float32)
        masks = masks.astype(bf16)

        qtok = np.concatenate(
            [np.arange(perm[pq] * QT, (perm[pq] + 1) * QT) for pq in Q_POS]
        )
        in_maps.append(dict(
            xT=xT, wqk=wqk_t, wv=wv_t, wo=wo_t,
            cos=cosp, sin=sinp,
            gq=gq_a, bq=bq_a, gk=gk_a, bk=bk_a, masks=masks,
            onesc=onesc, onesr=onesr, rotm=rotm,
        ))
        meta.append(dict(b=b, qtok=qtok))
    return in_maps, meta


_PROGRAM_CACHE = {}


def _get_program():
    if "full" not in _PROGRAM_CACHE:
        _PROGRAM_CACHE["full"] = build_program()
    return _PROGRAM_CACHE["full"]


def run_full(x, w_in, w_out, q_gamma, q_beta, k_gamma, k_beta,
             trace=False):
    from concourse.bass_utils import run_bass_kernel_spmd

    B = x.shape[0]
    n_cores = 2 * B
    in_maps, meta = make_host_data(
        x, w_in, w_out, q_gamma, q_beta, k_gamma, k_beta,
    )
    nc = _get_program()
    res = run_bass_kernel_spmd(
        nc, in_maps, core_ids=list(range(n_cores)), trace=trace,
    )
    out = np.empty((B, S, D), np.float32)
    for c in range(n_cores):
        o = res.results[c]["out"]  # [D, NQTOK]
        out[meta[c]["b"], meta[c]["qtok"], :] = o.T
    return out, res


def kernel(x, w_in, w_out, q_gamma, q_beta, k_gamma, k_beta, n_heads=16,
           **_ignored):
    x = np.asarray(x, np.float32)
    assert int(np.asarray(n_heads)) * HD == x.shape[-1]
    out, _ = run_full(
        np.asarray(x, np.float32),
        np.asarray(w_in, np.float32),
        np.asarray(w_out, np.float32),
        np.asarray(q_gamma, np.float32),
        np.asarray(q_beta, np.float32),
        np.asarray(k_gamma, np.float32),
        np.asarray(k_beta, np.float32),
    )
    return out


# revision 25
# speedup vs baseline: 1.3615x; 1.0735x over previous
"""Trainium2 Bass kernel for a custom attention block (qkv-proj + LN(q,k) +
RoPE + causal attention + out-proj), distributed over 8 NeuronCores.

Sharding: 2 cores per batch (B=4). Core role r=c%2 takes q-token blocks
{0,3} (r=0) or {1,2} (r=1) of 512 tokens; every core computes K/V for the
full 2048-token sequence of its batch (no collectives). The compiled
program is identical on all cores; per-core differences are input data
only. To keep the q-slab offsets compile-time-constant, each core sees
the sequence in a per-role BLOCK PERMUTATION (r=0: 0,1,2,3; r=1:
1,0,3,2), so its q blocks always sit at permuted positions {0,3}. The
cos/sin tables, causal masks and output assembly are permutation-aware
host data.

All matmuls run in bf16 (same PE rate as fp32r but faster weight loads
via fast-weight-load, half the DMA/SBUF), with fp32 PSUM accumulation.
x is SBUF-resident; q stays SBUF-resident post-rope; k/v round-trip
through DRAM scratch.

LN: mean subtraction is folded into host-pre-centered w_in rows;
variance comes from Square + ones-matmul partition reduction;
rsqrt(var+eps) = Exp(-0.5*Ln(var+eps)).
"""

import math

import numpy as np

import concourse.bass as bass
import concourse.mybir as mybir
import concourse.tile as tile
from concourse import bacc
from concourse.bass import ds

F32 = mybir.dt.float32
F32R = mybir.dt.float32r
BF16 = mybir.dt.bfloat16
AF = mybir.ActivationFunctionType
OP = mybir.AluOpType

P = 128
HD = 128
D = 2048
S = 2048
NH = D // HD          # 16 heads = feature chunks
DC = D // P           # 16 contraction chunks
NQTOK = 1024          # q tokens per core
QT = 512              # q/attention tile width (moving dim)
NQ = NQTOK // QT      # 2 q tiles per core
EXP_BIAS = 8.0
EPS = 1e-5
SLOTS = (8, 16)       # kv 128-chunks per q tile (max over the two roles)
# elementwise masks only on the diagonal slots (identical for both roles
# thanks to the block permutation); everything else is handled by the
# per-row exp bias (-EXP_BIAS valid / -30 invalid).
MASKED = ((0, 1, 2, 3), (12, 13, 14, 15))
MAXM = 4
BIAS_INVALID = -30.0
Q_POS = (0, 3)        # structural (permuted) block positions of q slabs
VET = 4               # v feature tiles of 512
VEW = 512
LOOKAHEAD = 2         # attention score-slot software pipeline depth


def _r(ap):
    """fp32 -> fp32r view for matmul operands."""
    return ap.bitcast(F32R)


def build_program():
    nc = bacc.Bacc("TRN2", target_bir_lowering=False, debug=False)

    # ---- I/O ----
    xT_i = nc.dram_tensor("xT", [D, S], BF16, kind="ExternalInput").ap()
    wqk_i = nc.dram_tensor("wqk", [2 * NH, P, DC, P], BF16,
                           kind="ExternalInput").ap()
    wv_i = nc.dram_tensor("wv", [VET, P, DC, VEW], BF16,
                          kind="ExternalInput").ap()
    wo_i = nc.dram_tensor("wo", [NH, P, NH, P], BF16,
                          kind="ExternalInput").ap()
    cos_i = nc.dram_tensor("cos", [HD, S], BF16, kind="ExternalInput").ap()
    sin_i = nc.dram_tensor("sin", [HD, S], BF16, kind="ExternalInput").ap()
    gq_i = nc.dram_tensor("gq", [P, NH], F32, kind="ExternalInput").ap()
    bq_i = nc.dram_tensor("bq", [P, NH], F32, kind="ExternalInput").ap()
    gk_i = nc.dram_tensor("gk", [P, NH], F32, kind="ExternalInput").ap()
    bk_i = nc.dram_tensor("bk", [P, NH], F32, kind="ExternalInput").ap()
    masks_i = nc.dram_tensor("masks", [NQ, P, MAXM, QT], BF16,
                             kind="ExternalInput").ap()
    biast_i = nc.dram_tensor("biast", [P, NQ, 16], F32,
                             kind="ExternalInput").ap()
    onesc_i = nc.dram_tensor("onesc", [P, 1], F32, kind="ExternalInput").ap()
    onesr_i = nc.dram_tensor("onesr", [1, P], F32, kind="ExternalInput").ap()
    rotm_i = nc.dram_tensor("rotm", [P, P], BF16, kind="ExternalInput").ap()
    out_t = nc.dram_tensor("out", [D, NQTOK], F32, kind="ExternalOutput").ap()

    with tile.TileContext(nc) as tc:
        import contextlib

        ctx = contextlib.ExitStack()
        with ctx:
            sb = ctx.enter_context(tc.tile_pool(name="sb", bufs=1))
            psum = ctx.enter_context(tc.tile_pool(name="ps", bufs=1, space="PSUM"))
            dram = ctx.enter_context(tc.tile_pool(name="dram", bufs=1, space="DRAM"))

            # ---- DRAM scratch ----
            kts = dram.tile([P, NH, S], BF16, tag="kts", name="kts")
            vs = dram.tile([VET, S, VEW], BF16, tag="vs", name="vs")

            # ---- constants / small inputs ----
            ones_col = sb.tile([P, 1], F32, tag="ones_col", name="ones_col")
            nc.sync.dma_start(_r(ones_col), _r(onesc_i))
            ones_row = sb.tile([1, P], F32, tag="ones_row", name="ones_row")
            nc.sync.dma_start(_r(ones_row), _r(onesr_i))
            eps1 = sb.tile([1, 1], F32, tag="eps1", name="eps1")
            nc.vector.memset(eps1, EPS)
            zero1 = sb.tile([1, 1], F32, tag="zero1", name="zero1")
            nc.vector.memset(zero1, 0.0)
            biast = sb.tile([P, NQ, 16], F32, tag="biast", name="biast")
            nc.sync.dma_start(biast, biast_i)
            rotm = sb.tile([P, P], BF16, tag="rotm", name="rotm")
            nc.sync.dma_start(rotm, rotm_i)
            gq = sb.tile([P, NH], F32, tag="gq", name="gq")
            nc.sync.dma_start(gq, gq_i)
            bq = sb.tile([P, NH], F32, tag="bq", name="bq")
            nc.sync.dma_start(bq, bq_i)
            gk = sb.tile([P, NH], F32, tag="gk", name="gk")
            nc.sync.dma_start(gk, gk_i)
            bk = sb.tile([P, NH], F32, tag="bk", name="bk")
            nc.sync.dma_start(bk, bk_i)
            cos_t = sb.tile([HD, S], BF16, tag="cos_t", name="cos_t")
            nc.sync.dma_start(cos_t, cos_i)
            sin_t = sb.tile([HD, S], BF16, tag="sin_t", name="sin_t")
            nc.sync.dma_start(sin_t, sin_i)

            # ---- resident x: [128, DC, S] bf16 (64KB/partition) ----
            xsb = sb.tile([P, DC, S], BF16, tag="xsb", bufs=1, name="xsb")
            for d in range(DC):
                nc.sync.dma_start(xsb[:, d], xT_i[ds(d * P, P), :])

            # ---- resident q (post-LN+rope): [128, NH, NQTOK] bf16 ----
            q_res = sb.tile([P, NH, NQTOK], BF16, tag="q_res", bufs=1,
                            name="q_res")

            def proj_group(ec_base, slab_offs, q_dst_offs, g_sb, b_sb):
                """Project x -> feature-partition [128, QT] tiles for each
                token slab in this group (2 slabs), with LN + rope.

                slab_offs: compile-time token offsets into the permuted
                sequence (index into x and cos/sin). If q_dst_offs is not
                None the result lands at q_res[:, :, q_dst_off]; else it
                DMAs to kts[:, :, slab_off].
                """
                n_s = len(slab_offs)
                holds = []
                for i in range(n_s):
                    if q_dst_offs is not None:
                        holds.append(q_res[:, :, ds(q_dst_offs[i], QT)])
                    else:
                        h = sb.tile([P, NH, QT], BF16, tag="khold", bufs=2,
                                    name="khold")
                        holds.append(h)
                sqsums = []
                for i in range(n_s):
                    sqsums.append(sb.tile([P, QT], F32, tag="acc", bufs=2,
                                          name="sqsum"))
                for ec in range(NH):
                    w = sb.tile([P, DC, P], BF16, tag="w", bufs=2, name="w")
                    nc.sync.dma_start(w, wqk_i[ec_base + ec])
                    pss = [psum.tile([P, QT], F32, tag="mm", bufs=5, name="ps")
                           for _ in range(n_s)]
                    for d in range(DC):
                        for i in range(n_s):
                            nc.tensor.matmul(
                                pss[i],
                                lhsT=w[:, d],
                                rhs=xsb[:, d, ds(slab_offs[i], QT)],
                                start=(d == 0),
                                stop=(d == DC - 1),
                            )
                    for i in range(n_s):
                        nc.vector.tensor_copy(holds[i][:, ec], pss[i])
                        sq = sb.tile([P, QT], BF16, tag="sq", bufs=2,
                                     name="sq")
                        nc.scalar.square(sq, pss[i])
                        if ec == 0:
                            nc.vector.tensor_copy(_r(sqsums[i]), sq)
                        else:
                            nc.vector.tensor_tensor(_r(sqsums[i]), sqsums[i],
                                                    sq, op=OP.add)
                for i in range(n_s):
                    hold = holds[i]
                    csl = ds(slab_offs[i], QT)
                    # per-token sumsq: partition-sum of sqsum via ones-matmul
                    pstat = psum.tile([1, QT], F32, tag="stat", bufs=3,
                                      name="pstat")
                    nc.tensor.matmul(pstat, lhsT=_r(ones_col),
                                     rhs=_r(sqsums[i]))
                    # rsig = exp(-0.5 * ln(sumsq/D + eps))
                    lnv = sb.tile([1, QT], F32, tag="stats_sb", bufs=4,
                                  name="lnv")
                    nc.scalar.activation(lnv, pstat, AF.Ln,
                                         scale=1.0 / D, bias=eps1)
                    rsig = sb.tile([1, QT], F32, tag="stats_sb", bufs=4,
                                   name="rsig")
                    nc.scalar.activation(_r(rsig), lnv, AF.Exp, bias=zero1,
                                         scale=-0.5)
                    ps_rep = psum.tile([P, QT], F32, tag="mm", bufs=5,
                                       name="ps_rep")
                    nc.tensor.matmul(ps_rep, lhsT=_r(ones_row), rhs=_r(rsig))
                    # pass 1: LN apply on all chunks (DVE)
                    for ec in range(NH):
                        ch = hold[:, ec]
                        nc.vector.tensor_tensor(ch, ch, ps_rep, op=OP.mult)
                        nc.vector.tensor_scalar(
                            ch, ch,
                            scalar1=g_sb[:, ds(ec, 1)],
                            scalar2=b_sb[:, ds(ec, 1)],
                            op0=OP.mult, op1=OP.add,
                        )
                    # pass 2: rope rotation matmuls stream back-to-back
                    for ec in range(NH):
                        ch = hold[:, ec]
                        ps_rot = psum.tile([P, QT], F32, tag="mm", bufs=5,
                                           name="ps_rot")
                        nc.tensor.matmul(ps_rot, lhsT=rotm, rhs=ch)
                        tmp = sb.tile([P, QT], BF16, tag="rtmp", bufs=2,
                                      name="rtmp")
                        nc.vector.tensor_tensor(tmp, ps_rot, sin_t[:, csl],
                                                op=OP.mult)
                        nc.vector.tensor_tensor(ch, ch, cos_t[:, csl],
                                                op=OP.mult)
                        nc.vector.tensor_tensor(ch, ch, tmp, op=OP.add)
                    if q_dst_offs is None:
                        nc.sync.dma_start(
                            kts[:, :, ds(slab_offs[i], QT)], hold
                        )

            # ---- Phase Q: q projection (permuted positions 0 and 3) ----
            proj_group(0, [Q_POS[0] * QT, Q_POS[1] * QT], [0, QT], gq, bq)

            # ---- Phase K: 2 groups of 2 slabs ----
            proj_group(NH, [0, QT], None, gk, bk)
            proj_group(NH, [2 * QT, 3 * QT], None, gk, bk)

            # ---- Phase V ----
            KC = S // P  # 16 kv chunks
            for et in range(VET):
                for scg in range(KC // 2):
                    scs = [scg * 2, scg * 2 + 1]
                    psv = {sc: psum.tile([P, VEW], F32, tag="mm", bufs=5,
                                         name="psv")
                           for sc in scs}
                    for d in range(DC):
                        wvt = sb.tile([P, VEW], BF16, tag="wvt", bufs=4,
                                      name="wvt")
                        nc.scalar.dma_start(wvt, wv_i[et, :, d])
                        for sc in scs:
                            nc.tensor.matmul(
                                psv[sc],
                                lhsT=xsb[:, d, ds(sc * P, P)],
                                rhs=wvt,
                                start=(d == 0),
                                stop=(d == DC - 1),
                            )
                    for sc in scs:
                        vsb = sb.tile([P, VEW], BF16, tag="vsb", bufs=3,
                                      name="vsb")
                        nc.vector.tensor_copy(vsb, psv[sc])
                        nc.gpsimd.dma_start(vs[et, ds(sc * P, P), :], vsb)

            # ---- Attention + out-projection per q tile ----
            for t in range(NQ):
                qsl_off = t * QT
                n_slots = SLOTS[t]
                # masks overlay the (now dead) cos buffer
                mt = sb.tile([P, MAXM, QT], BF16, tag="cos_t",
                             bufs=1, name="mt")
                nc.sync.dma_start(mt, masks_i[t])
                mpos = {kc: i for i, kc in enumerate(MASKED[t])}
                ot_res = sb.tile([P, NH, QT], BF16, tag="khold", bufs=2,
                                 name="ot_res")
                for h in range(NH):
                    ksl = sb.tile([P, S], BF16, tag="kslab", bufs=2,
                                  name="ksl")
                    nc.sync.dma_start(ksl[:, ds(0, n_slots * P)],
                                      kts[:, h, ds(0, n_slots * P)])
                    vsl = sb.tile([P, KC, HD], BF16, tag="vslab", bufs=2,
                                  name="vsl")
                    nc.sync.dma_start(
                        vsl[:, ds(0, n_slots)],
                        vs[h // 4, ds(0, n_slots * P),
                           ds((h % 4) * HD, HD)].rearrange(
                            "(kc p) hd -> p kc hd", p=P
                        ),
                    )
                    psout = psum.tile([P, QT], F32, tag="mm", bufs=5,
                                      name="psout")
                    esum = sb.tile([P, QT], F32, tag="acc", bufs=2,
                                   name="esum")
                    esum_b = sb.tile([P, QT], F32, tag="sin_t", bufs=1,
                                     name="esum_b")
                    qsl = q_res[:, h, ds(qsl_off, QT)]

                    ets = {}

                    def emit_score(s):
                        pss = psum.tile([P, QT], F32, tag="mm", bufs=5,
                                        name="pss")
                        nc.tensor.matmul(pss, lhsT=ksl[:, ds(s * P, P)],
                                         rhs=qsl)
                        et = sb.tile([P, QT], BF16, tag="exp", bufs=4,
                                     name="et")
                        nc.scalar.activation(et, pss, AF.Exp,
                                             bias=biast[:, t, ds(s, 1)])
                        if s in mpos:
                            nc.vector.tensor_tensor(et, et, mt[:, mpos[s]],
                                                    op=OP.mult)
                        ets[s] = et

                    for s in range(min(LOOKAHEAD, n_slots)):
                        emit_score(s)
                    for s in range(n_slots):
                        if s + LOOKAHEAD < n_slots:
                            emit_score(s + LOOKAHEAD)
                        et = ets.pop(s)
                        nc.tensor.matmul(
                            psout,
                            lhsT=vsl[:, s],
                            rhs=et,
                            start=(s == 0),
                            stop=(s == n_slots - 1),
                        )
                        # denominator accumulation off the PE: two parallel
                        # chains (even slots on DVE, odd slots on GpSimd)
                        if s == 0:
                            nc.vector.tensor_copy(_r(esum), et)
                        elif s == 1:
                            nc.vector.tensor_copy(_r(esum_b), et)
                        elif s % 2 == 0:
                            nc.vector.tensor_tensor(_r(esum), esum, et,
                                                    op=OP.add)
                        else:
                            nc.gpsimd.tensor_tensor(_r(esum_b), esum_b, et,
                                                    op=OP.add)
                    nc.vector.tensor_tensor(_r(esum), esum, esum_b, op=OP.add)
                    psden = psum.tile([1, QT], F32, tag="stat", bufs=3,
                                      name="psden")
                    nc.tensor.matmul(psden, lhsT=_r(ones_col), rhs=_r(esum))
                    rec0 = sb.tile([1, QT], F32, tag="stats_sb", bufs=4,
                                   name="rec0")
                    with nc.allow_low_precision(
                        reason="denominator reciprocal, 18 bits is plenty"
                    ):
                        nc.vector.reciprocal_approx_fast(rec0, psden)
                    rec = sb.tile([1, QT], F32, tag="stats_sb", bufs=4,
                                  name="rec")
                    nc.scalar.activation(_r(rec), rec0, AF.Copy)
                    psr = psum.tile([P, QT], F32, tag="mm", bufs=5,
                                    name="psr")
                    nc.tensor.matmul(psr, lhsT=_r(ones_row), rhs=_r(rec))
                    nc.vector.tensor_copy(ot_res[:, h], psout)
                    nc.vector.tensor_tensor(ot_res[:, h], ot_res[:, h], psr,
                                            op=OP.mult)

                # ---- out-projection for this q tile ----
                for e in range(NH):
                    wot = sb.tile([P, NH, P], BF16, tag="wot", bufs=2,
                                  name="wot")
                    nc.sync.dma_start(wot, wo_i[e])
                    psf = psum.tile([P, QT], F32, tag="mm", bufs=5,
                                    name="psf")
                    for h in range(NH):
                        nc.tensor.matmul(
                            psf,
                            lhsT=wot[:, h],
                            rhs=ot_res[:, h],
                            start=(h == 0),
                            stop=(h == NH - 1),
                        )
                    fsb = sb.tile([P, QT], F32, tag="fsb", bufs=2,
                                  name="fsb")
                    nc.vector.tensor_copy(fsb, psf)
                    nc.sync.dma_start(
                        out_t[ds(e * P, P), ds(qsl_off, QT)], fsb
                    )

    nc.compile()
    return nc


# --------------------------------------------------------------------------
# Host-side prep and driver
# --------------------------------------------------------------------------

_PERMS = {0: (0, 1, 2, 3), 1: (1, 0, 3, 2)}


def make_host_data(x, w_in, w_out, q_gamma, q_beta, k_gamma, k_beta):
    """Build per-core in_maps (list of dicts) + assembly metadata."""
    import ml_dtypes
    bf16 = ml_dtypes.bfloat16

    B = x.shape[0]
    n_cores = 2 * B

    w64 = np.asarray(w_in, np.float64)
    wq = w64[0:D]
    wk = w64[D:2 * D]
    wv = w64[2 * D:3 * D]
    wq_c = wq - wq.mean(axis=0, keepdims=True)
    wk_c = wk - wk.mean(axis=0, keepdims=True)
    wqkT2 = np.concatenate([wq_c.T, wk_c.T], axis=1)   # [D, 2D]
    wqk_t = np.ascontiguousarray(
        wqkT2.reshape(DC, P, 2 * NH, P).transpose(2, 1, 0, 3)
    ).astype(bf16)
    wvT = wv.T  # [D, D]
    wv_t = np.ascontiguousarray(
        wvT.reshape(DC, P, VET, VEW).transpose(2, 1, 0, 3)
    ).astype(bf16)
    woT = np.asarray(w_out, np.float64).T  # [D(hfeat), D(eout)]
    wo_t = np.ascontiguousarray(
        woT.reshape(NH, P, NH, P).transpose(2, 1, 0, 3)
    ).astype(bf16)

    inv = 1.0 / (10000.0 ** (np.arange(0, HD, 2, dtype=np.float64) / HD))
    tpos = np.arange(S, dtype=np.float64)
    fr = np.outer(tpos, inv)
    emb = np.concatenate([fr, fr], axis=-1)  # [S, HD]
    cosT = np.cos(emb).T  # [HD, S]
    sinT = np.sin(emb).T

    h2 = HD // 2
    rotmT = np.zeros((P, P), np.float32)
    for p in range(h2):
        rotmT[p + h2, p] = -1.0
    for p in range(h2, HD):
        rotmT[p - h2, p] = 1.0
    rotm = rotmT.astype(bf16)

    scale = 1.0 / math.sqrt(HD)
    gq_a = np.ascontiguousarray(
        (np.asarray(q_gamma, np.float64) * scale).reshape(NH, P).T
    ).astype(np.float32)
    bq_a = np.ascontiguousarray(
        (np.asarray(q_beta, np.float64) * scale).reshape(NH, P).T
    ).astype(np.float32)
    gk_a = np.ascontiguousarray(
        np.asarray(k_gamma, np.float32).reshape(NH, P).T
    )
    bk_a = np.ascontiguousarray(
        np.asarray(k_beta, np.float32).reshape(NH, P).T
    )
    onesc = np.ones((P, 1), np.float32)
    onesr = np.ones((1, P), np.float32)

    xb_T = {}
    in_maps = []
    meta = []
    for c in range(n_cores):
        b = c // 2
        r = c % 2
        perm = _PERMS[r]
        ptok = np.concatenate(
            [np.arange(pb * QT, (pb + 1) * QT) for pb in perm]
        )
        if b not in xb_T:
            xb_T[b] = np.ascontiguousarray(
                np.asarray(x[b], np.float32).T
            )  # [D, S] f32
        xT = np.ascontiguousarray(xb_T[b][:, ptok]).astype(bf16)
        cosp = np.ascontiguousarray(cosT[:, ptok]).astype(bf16)
        sinp = np.ascontiguousarray(sinT[:, ptok]).astype(bf16)

        # masks in PERMUTED kv space; q slabs at permuted positions Q_POS.
        # Elementwise masks only on diagonal slots; other slots use the
        # per-row exp bias: -EXP_BIAS for fully valid rows, BIAS_INVALID
        # for fully invalid rows.
        masks = np.zeros([NQ, P, MAXM, QT], np.float32)
        biast = np.full([P, NQ, 16], -EXP_BIAS, np.float32)
        for t in range(NQ):
            gq_tok = ptok[Q_POS[t] * QT + np.arange(QT)]
            gq_max = gq_tok.max()
            for mi, kc in enumerate(MASKED[t]):
                gkv = ptok[kc * P + np.arange(P)]
                masks[t, :, mi, :] = (
                    gkv[:, None] <= gq_tok[None, :]
                ).astype(np.float32)
            for kc in range(16):
                gkv = ptok[kc * P + np.arange(P)]
                biast[:, t, kc] = np.where(gkv <= gq_max, -EXP_BIAS,
                                           BIAS_INVALID)
        masks = masks.astype(bf16)

        qtok = np.concatenate(
            [np.arange(perm[pq] * QT, (perm[pq] + 1) * QT) for pq in Q_POS]
        )
        in_maps.append(dict(
            xT=xT, wqk=wqk_t, wv=wv_t, wo=wo_t,
            cos=cosp, sin=sinp,
            gq=gq_a, bq=bq_a, gk=gk_a, bk=bk_a, masks=masks,
            biast=biast, onesc=onesc, onesr=onesr, rotm=rotm,
        ))
        meta.append(dict(b=b, qtok=qtok))
    return in_maps, meta


_PROGRAM_CACHE = {}


def _get_program():
    if "full" not in _PROGRAM_CACHE:
        _PROGRAM_CACHE["full"] = build_program()
    return _PROGRAM_CACHE["full"]


def run_full(x, w_in, w_out, q_gamma, q_beta, k_gamma, k_beta,
             trace=False):
    from concourse.bass_utils import run_bass_kernel_spmd

    B = x.shape[0]
    n_cores = 2 * B
    in_maps, meta = make_host_data(
        x, w_in, w_out, q_gamma, q_beta, k_gamma, k_beta,
    )
    nc = _get_program()
    res = run_bass_kernel_spmd(
        nc, in_maps, core_ids=list(range(n_cores)), trace=trace,
    )
    out = np.empty((B, S, D), np.float32)
    for c in range(n_cores):
        o = res.results[c]["out"]  # [D, NQTOK]
        out[meta[c]["b"], meta[c]["qtok"], :] = o.T
    return out, res


def kernel(x, w_in, w_out, q_gamma, q_beta, k_gamma, k_beta, n_heads=16,
           **_ignored):
    x = np.asarray(x, np.float32)
    assert int(np.asarray(n_heads)) * HD == x.shape[-1]
    out, _ = run_full(
        np.asarray(x, np.float32),
        np.asarray(w_in, np.float32),
        np.asarray(w_out, np.float32),
        np.asarray(q_gamma, np.float32),
        np.asarray(q_beta, np.float32),
        np.asarray(k_gamma, np.float32),
        np.asarray(k_beta, np.float32),
    )
    return out


# revision 27
# speedup vs baseline: 1.5866x; 1.1653x over previous
"""Trainium2 Bass kernel for a custom attention block (qkv-proj + LN(q,k) +
RoPE + causal attention + out-proj), distributed over 8 NeuronCores.

Sharding: 2 cores per batch (B=4). Core role r=c%2 takes q-token blocks
{0,3} (r=0) or {1,2} (r=1) of 512 tokens; every core computes K/V for the
full 2048-token sequence of its batch (no collectives). The compiled
program is identical on all cores; per-core differences are input data
only. To keep the q-slab offsets compile-time-constant, each core sees
the sequence in a per-role BLOCK PERMUTATION (r=0: 0,1,2,3; r=1:
1,0,3,2), so its q blocks always sit at permuted positions {0,3} and the
causal diagonal lands on the same slot indices for both roles. The
cos/sin tables, causal masks, exp row-biases and output assembly are
permutation-aware host data.

All matmuls run in bf16 (same PE rate as fp32r but faster weight loads,
half the DMA/SBUF), with fp32 PSUM accumulation. x is SBUF-resident; q
stays SBUF-resident post-rope; k round-trips through DRAM feature-major;
v is computed feature-major (so the PE reuses each stationary weight
tile across 4 moving tiles) and transposed to token-major on the fly by
DMA-transpose loads during attention.

Engine split: PE does projections/scores/PV; Scalar does exp, squares
and PSUM->SBUF copies; DVE does LN/rope muls, masks and half the
softmax-denominator accumulation; GpSimd does the other half plus the
rope add and LN sumsq chains. Softmax denominator = chained elementwise
adds of the exp tiles + one ones-matmul partition reduction per head.
"""

import math

import numpy as np

import concourse.bass as bass
import concourse.mybir as mybir
import concourse.tile as tile
from concourse import bacc
from concourse.bass import ds

F32 = mybir.dt.float32
F32R = mybir.dt.float32r
BF16 = mybir.dt.bfloat16
AF = mybir.ActivationFunctionType
OP = mybir.AluOpType

P = 128
HD = 128
D = 2048
S = 2048
NH = D // HD          # 16 heads = feature chunks
DC = D // P           # 16 contraction chunks
NQTOK = 1024          # q tokens per core
QT = 512              # q/attention tile width (moving dim)
NQ = NQTOK // QT      # 2 q tiles per core
EXP_BIAS = 8.0
EPS = 1e-5
SLOTS = (8, 16)       # kv 128-chunks per q tile (max over the two roles)
# elementwise masks only on the diagonal slots (identical for both roles
# thanks to the block permutation); everything else is handled by the
# per-row exp bias (-EXP_BIAS valid / BIAS_INVALID invalid).
MASKED = ((0, 1, 2, 3), (12, 13, 14, 15))
MAXM = 4
BIAS_INVALID = -30.0
Q_POS = (0, 3)        # structural (permuted) block positions of q slabs
KC = S // P           # 16 kv chunks
LOOKAHEAD = 2         # attention score-slot software pipeline depth


def _r(ap):
    """fp32 -> fp32r view for matmul operands."""
    return ap.bitcast(F32R)


def _v3(ap):
    """[P, n*128] AP -> [P, n, 128] view (avoids 1-free-dim DMA splits)."""
    return ap.rearrange("p (a x) -> p a x", x=P)


def build_program():
    nc = bacc.Bacc("TRN2", target_bir_lowering=False, debug=False)

    # ---- I/O ----
    xT_i = nc.dram_tensor("xT", [D, S], BF16, kind="ExternalInput").ap()
    wqk_i = nc.dram_tensor("wqk", [2 * NH, P, DC, P], BF16,
                           kind="ExternalInput").ap()
    wv_i = nc.dram_tensor("wv", [DC, P, NH, P], BF16,
                          kind="ExternalInput").ap()
    wo_i = nc.dram_tensor("wo", [NH, P, NH, P], BF16,
                          kind="ExternalInput").ap()
    cos_i = nc.dram_tensor("cos", [HD, S], BF16, kind="ExternalInput").ap()
    sin_i = nc.dram_tensor("sin", [HD, S], BF16, kind="ExternalInput").ap()
    gq_i = nc.dram_tensor("gq", [P, NH], F32, kind="ExternalInput").ap()
    bq_i = nc.dram_tensor("bq", [P, NH], F32, kind="ExternalInput").ap()
    gk_i = nc.dram_tensor("gk", [P, NH], F32, kind="ExternalInput").ap()
    bk_i = nc.dram_tensor("bk", [P, NH], F32, kind="ExternalInput").ap()
    masks_i = nc.dram_tensor("masks", [NQ, P, MAXM, QT], BF16,
                             kind="ExternalInput").ap()
    biast_i = nc.dram_tensor("biast", [P, NQ, 16], F32,
                             kind="ExternalInput").ap()
    onesc_i = nc.dram_tensor("onesc", [P, 1], F32, kind="ExternalInput").ap()
    onesr_i = nc.dram_tensor("onesr", [1, P], F32, kind="ExternalInput").ap()
    rotm_i = nc.dram_tensor("rotm", [P, P], BF16, kind="ExternalInput").ap()
    out_t = nc.dram_tensor("out", [D, NQTOK], F32, kind="ExternalOutput").ap()

    with tile.TileContext(nc) as tc:
        import contextlib

        ctx = contextlib.ExitStack()
        with ctx:
            sb = ctx.enter_context(tc.tile_pool(name="sb", bufs=1))
            psum = ctx.enter_context(tc.tile_pool(name="ps", bufs=1, space="PSUM"))
            dram = ctx.enter_context(tc.tile_pool(name="dram", bufs=1, space="DRAM"))

            # ---- DRAM scratch ----
            kts = dram.tile([P, NH, S], BF16, tag="kts", name="kts")
            vT = dram.tile([D, S], BF16, tag="vT", name="vT")

            # ---- constants / small inputs ----
            ones_col = sb.tile([P, 1], F32, tag="ones_col", name="ones_col")
            nc.sync.dma_start(_r(ones_col), _r(onesc_i))
            ones_row = sb.tile([1, P], F32, tag="ones_row", name="ones_row")
            nc.sync.dma_start(_r(ones_row), _r(onesr_i))
            eps1 = sb.tile([1, 1], F32, tag="eps1", name="eps1")
            nc.vector.memset(eps1, EPS)
            zero1 = sb.tile([1, 1], F32, tag="zero1", name="zero1")
            nc.vector.memset(zero1, 0.0)
            biast = sb.tile([P, NQ, 16], F32, tag="biast", name="biast")
            nc.sync.dma_start(biast, biast_i)
            rotm = sb.tile([P, P], BF16, tag="rotm", name="rotm")
            nc.sync.dma_start(rotm, rotm_i)
            gq = sb.tile([P, NH], F32, tag="gq", name="gq")
            nc.sync.dma_start(gq, gq_i)
            bq = sb.tile([P, NH], F32, tag="bq", name="bq")
            nc.sync.dma_start(bq, bq_i)
            gk = sb.tile([P, NH], F32, tag="gk", name="gk")
            nc.sync.dma_start(gk, gk_i)
            bk = sb.tile([P, NH], F32, tag="bk", name="bk")
            nc.sync.dma_start(bk, bk_i)
            cos_t = sb.tile([HD, S], BF16, tag="cos_t", name="cos_t")
            nc.sync.dma_start(_v3(cos_t), _v3(cos_i))
            sin_t = sb.tile([HD, S], BF16, tag="sin_t", name="sin_t")
            nc.sync.dma_start(_v3(sin_t), _v3(sin_i))

            # ---- resident x: [128, DC, S] bf16 (64KB/partition) ----
            xsb = sb.tile([P, DC, S], BF16, tag="xsb", bufs=1, name="xsb")
            for d in range(DC):
                nc.sync.dma_start(xsb[:, d], xT_i[ds(d * P, P), :])

            # ---- resident q (post-LN+rope): [128, NH, NQTOK] bf16 ----
            q_res = sb.tile([P, NH, NQTOK], BF16, tag="q_res", bufs=1,
                            name="q_res")

            def proj_group(ec_base, slab_offs, q_dst_offs, g_sb, b_sb):
                """Project x -> feature-partition [128, QT] tiles for each
                token slab in this group, with LN + rope.

                slab_offs: compile-time token offsets into the permuted
                sequence (index into x and cos/sin). If q_dst_offs is not
                None the result lands at q_res[:, :, q_dst_off]; else it
                DMAs to kts[:, :, slab_off].
                """
                n_s = len(slab_offs)
                holds = []
                for i in range(n_s):
                    if q_dst_offs is not None:
                        holds.append(q_res[:, :, ds(q_dst_offs[i], QT)])
                    else:
                        h = sb.tile([P, NH, QT], BF16, tag="khold", bufs=2,
                                    name="khold")
                        holds.append(h)
                sqsums = []
                for i in range(n_s):
                    sqsums.append(sb.tile([P, QT], F32, tag="acc", bufs=2,
                                          name="sqsum"))
                for ec in range(NH):
                    w = sb.tile([P, DC, P], BF16, tag="w", bufs=2, name="w")
                    nc.sync.dma_start(w, wqk_i[ec_base + ec])
                    pss = [psum.tile([P, QT], F32, tag="mm", bufs=5, name="ps")
                           for _ in range(n_s)]
                    for d in range(DC):
                        for i in range(n_s):
                            nc.tensor.matmul(
                                pss[i],
                                lhsT=w[:, d],
                                rhs=xsb[:, d, ds(slab_offs[i], QT)],
                                start=(d == 0),
                                stop=(d == DC - 1),
                            )
                    for i in range(n_s):
                        nc.scalar.copy(holds[i][:, ec], pss[i])
                        sq = sb.tile([P, QT], BF16, tag="sq", bufs=2,
                                     name="sq")
                        nc.scalar.square(sq, pss[i])
                        if ec == 0:
                            nc.vector.tensor_copy(_r(sqsums[i]), sq)
                        else:
                            nc.gpsimd.tensor_tensor(_r(sqsums[i]), sqsums[i],
                                                    sq, op=OP.add)
                for i in range(n_s):
                    hold = holds[i]
                    csl = ds(slab_offs[i], QT)
                    # per-token sumsq: partition-sum of sqsum via ones-matmul
                    pstat = psum.tile([1, QT], F32, tag="stat", bufs=3,
                                      name="pstat")
                    nc.tensor.matmul(pstat, lhsT=_r(ones_col),
                                     rhs=_r(sqsums[i]))
                    # rsig = exp(-0.5 * ln(sumsq/D + eps))
                    lnv = sb.tile([1, QT], F32, tag="stats_sb", bufs=4,
                                  name="lnv")
                    nc.scalar.activation(lnv, pstat, AF.Ln,
                                         scale=1.0 / D, bias=eps1)
                    rsig = sb.tile([1, QT], F32, tag="stats_sb", bufs=4,
                                   name="rsig")
                    nc.scalar.activation(_r(rsig), lnv, AF.Exp, bias=zero1,
                                         scale=-0.5)
                    ps_rep = psum.tile([P, QT], F32, tag="mm", bufs=5,
                                       name="ps_rep")
                    nc.tensor.matmul(ps_rep, lhsT=_r(ones_row), rhs=_r(rsig))
                    # pass 1: LN apply on all chunks (DVE)
                    for ec in range(NH):
                        ch = hold[:, ec]
                        nc.vector.tensor_tensor(ch, ch, ps_rep, op=OP.mult)
                        nc.vector.tensor_scalar(
                            ch, ch,
                            scalar1=g_sb[:, ds(ec, 1)],
                            scalar2=b_sb[:, ds(ec, 1)],
                            op0=OP.mult, op1=OP.add,
                        )
                    # pass 2: rope; rotation matmuls stream back-to-back
                    for ec in range(NH):
                        ch = hold[:, ec]
                        ps_rot = psum.tile([P, QT], F32, tag="mm", bufs=5,
                                           name="ps_rot")
                        nc.tensor.matmul(ps_rot, lhsT=rotm, rhs=ch)
                        tmp = sb.tile([P, QT], BF16, tag="rtmp", bufs=2,
                                      name="rtmp")
                        nc.vector.tensor_tensor(tmp, ps_rot, sin_t[:, csl],
                                                op=OP.mult)
                        nc.vector.tensor_tensor(ch, ch, cos_t[:, csl],
                                                op=OP.mult)
                        nc.gpsimd.tensor_tensor(ch, ch, tmp, op=OP.add)
                    if q_dst_offs is None:
                        nc.sync.dma_start(
                            kts[:, :, ds(slab_offs[i], QT)], hold
                        )

            # ---- Phase Q: q projection (permuted positions 0 and 3) ----
            proj_group(0, [Q_POS[0] * QT, Q_POS[1] * QT], [0, QT], gq, bq)

            # ---- Phase K: 4 single-slab groups (overlap LN/rope tails) ----
            for g in range(4):
                proj_group(NH, [g * QT], None, gk, bk)

            # ---- Phase V: w-stationary (1 LDW per 4 matmuls), vT output ----
            for f in range(NH):
                wvf = sb.tile([P, DC, P], BF16, tag="w", bufs=2, name="wvf")
                nc.sync.dma_start(wvf, wv_i[:, :, f, :].rearrange(
                    "d p j -> p d j"))
                psv = [psum.tile([P, QT], F32, tag="mm", bufs=5, name="psv")
                       for _ in range(4)]
                for d in range(DC):
                    for ts in range(4):
                        nc.tensor.matmul(
                            psv[ts],
                            lhsT=wvf[:, d],
                            rhs=xsb[:, d, ds(ts * QT, QT)],
                            start=(d == 0),
                            stop=(d == DC - 1),
                        )
                for ts in range(4):
                    vtsb = sb.tile([P, QT], BF16, tag="vsb", bufs=3,
                                   name="vtsb")
                    nc.scalar.copy(vtsb, psv[ts])
                    nc.gpsimd.dma_start(
                        _v3(vT[ds(f * P, P), ds(ts * QT, QT)]), _v3(vtsb)
                    )

            # ---- Attention + out-projection per q tile ----
            for t in range(NQ):
                qsl_off = t * QT
                n_slots = SLOTS[t]
                # masks overlay the (now dead) cos buffer
                mt = sb.tile([P, MAXM, QT], BF16, tag="cos_t", bufs=1,
                             name="mt")
                nc.sync.dma_start(mt, masks_i[t])
                mpos = {kc: i for i, kc in enumerate(MASKED[t])}
                ot_res = sb.tile([P, NH, QT], BF16, tag="khold", bufs=2,
                                 name="ot_res")
                pending = None

                def finish_norm(pending):
                    psout_p, esum_p, h_p = pending
                    psden = psum.tile([1, QT], F32, tag="stat", bufs=3,
                                      name="psden")
                    nc.tensor.matmul(psden, lhsT=_r(ones_col), rhs=_r(esum_p))
                    rec0 = sb.tile([1, QT], F32, tag="stats_sb", bufs=4,
                                   name="rec0")
                    with nc.allow_low_precision(
                        reason="denominator reciprocal, 18 bits is plenty"
                    ):
                        nc.vector.reciprocal_approx_fast(rec0, psden)
                    rec = sb.tile([1, QT], F32, tag="stats_sb", bufs=4,
                                  name="rec")
                    nc.vector.tensor_copy(_r(rec), rec0)
                    psr = psum.tile([P, QT], F32, tag="mm", bufs=5,
                                    name="psr")
                    nc.tensor.matmul(psr, lhsT=_r(ones_row), rhs=_r(rec))
                    nc.vector.tensor_copy(ot_res[:, h_p], psout_p)
                    nc.vector.tensor_tensor(ot_res[:, h_p], ot_res[:, h_p],
                                            psr, op=OP.mult)

                for h in range(NH):
                    ksl = sb.tile([P, KC, P], BF16, tag="kslab", bufs=2,
                                  name="ksl")
                    nc.sync.dma_start(
                        ksl[:, ds(0, n_slots)],
                        kts[:, h].rearrange("p (c x) -> p c x", x=P)[
                            :, ds(0, n_slots)],
                    )
                    vsl = sb.tile([P, KC, HD], BF16, tag="vslab", bufs=2,
                                  name="vsl")
                    nc.sync.dma_start_transpose(
                        vsl[:, ds(0, n_slots)],
                        vT[ds(h * HD, HD), ds(0, n_slots * P)],
                    )
                    psout = psum.tile([P, QT], F32, tag="mm", bufs=5,
                                      name="psout")
                    esum = sb.tile([P, QT], F32, tag="acc", bufs=2,
                                   name="esum")
                    esum_b = sb.tile([P, QT], F32, tag="sin_t", bufs=1,
                                     name="esum_b")
                    qsl = q_res[:, h, ds(qsl_off, QT)]

                    ets = {}

                    def emit_score(s):
                        pss = psum.tile([P, QT], F32, tag="mm", bufs=5,
                                        name="pss")
                        nc.tensor.matmul(pss, lhsT=ksl[:, s], rhs=qsl)
                        et = sb.tile([P, QT], BF16, tag="exp", bufs=4,
                                     name="et")
                        nc.scalar.activation(et, pss, AF.Exp,
                                             bias=biast[:, t, ds(s, 1)])
                        if s in mpos:
                            nc.vector.tensor_tensor(et, et, mt[:, mpos[s]],
                                                    op=OP.mult)
                        ets[s] = et

                    for s in range(min(LOOKAHEAD, n_slots)):
                        emit_score(s)
                    # previous head's normalization, pipelined behind our
                    # prologue so the PE never waits on its denominator
                    if pending is not None:
                        finish_norm(pending)
                    for s in range(n_slots):
                        if s + LOOKAHEAD < n_slots:
                            emit_score(s + LOOKAHEAD)
                        et = ets.pop(s)
                        nc.tensor.matmul(
                            psout,
                            lhsT=vsl[:, s],
                            rhs=et,
                            start=(s == 0),
                            stop=(s == n_slots - 1),
                        )
                        # denominator accumulation off the PE: two parallel
                        # chains (even slots on DVE, odd slots on GpSimd)
                        if s == 0:
                            nc.vector.tensor_copy(_r(esum), et)
                        elif s == 1:
                            nc.vector.tensor_copy(_r(esum_b), et)
                        elif s % 2 == 0:
                            nc.vector.tensor_tensor(_r(esum), esum, et,
                                                    op=OP.add)
                        else:
                            nc.gpsimd.tensor_tensor(_r(esum_b), esum_b, et,
                                                    op=OP.add)
                    nc.vector.tensor_tensor(_r(esum), esum, esum_b, op=OP.add)
                    pending = (psout, esum, h)
                finish_norm(pending)

                # ---- out-projection for this q tile ----
                for e in range(NH):
                    wot = sb.tile([P, NH, P], BF16, tag="wot", bufs=2,
                                  name="wot")
                    nc.sync.dma_start(wot, wo_i[e])
                    psf = psum.tile([P, QT], F32, tag="mm", bufs=5,
                                    name="psf")
                    for h in range(NH):
                        nc.tensor.matmul(
                            psf,
                            lhsT=wot[:, h],
                            rhs=ot_res[:, h],
                            start=(h == 0),
                            stop=(h == NH - 1),
                        )
                    fsb = sb.tile([P, QT], F32, tag="fsb", bufs=2,
                                  name="fsb")
                    nc.vector.tensor_copy(fsb, psf)
                    nc.sync.dma_start(
                        _v3(out_t[ds(e * P, P), ds(qsl_off, QT)]), _v3(fsb)
                    )

    nc.compile()
    return nc


# --------------------------------------------------------------------------
# Host-side prep and driver
# --------------------------------------------------------------------------

_PERMS = {0: (0, 1, 2, 3), 1: (1, 0, 3, 2)}


def make_host_data(x, w_in, w_out, q_gamma, q_beta, k_gamma, k_beta):
    """Build per-core in_maps (list of dicts) + assembly metadata."""
    import ml_dtypes
    bf16 = ml_dtypes.bfloat16

    B = x.shape[0]
    n_cores = 2 * B

    w64 = np.asarray(w_in, np.float64)
    wq = w64[0:D]
    wk = w64[D:2 * D]
    wv = w64[2 * D:3 * D]
    wq_c = wq - wq.mean(axis=0, keepdims=True)
    wk_c = wk - wk.mean(axis=0, keepdims=True)
    wqkT2 = np.concatenate([wq_c.T, wk_c.T], axis=1)   # [D, 2D]
    wqk_t = np.ascontiguousarray(
        wqkT2.reshape(DC, P, 2 * NH, P).transpose(2, 1, 0, 3)
    ).astype(bf16)
    wvT = wv.T  # [D(d), D(f)]
    wv_t = np.ascontiguousarray(
        wvT.reshape(DC, P, NH, P)
    ).astype(bf16)
    woT = np.asarray(w_out, np.float64).T  # [D(hfeat), D(eout)]
    wo_t = np.ascontiguousarray(
        woT.reshape(NH, P, NH, P).transpose(2, 1, 0, 3)
    ).astype(bf16)

    inv = 1.0 / (10000.0 ** (np.arange(0, HD, 2, dtype=np.float64) / HD))
    tpos = np.arange(S, dtype=np.float64)
    fr = np.outer(tpos, inv)
    emb = np.concatenate([fr, fr], axis=-1)  # [S, HD]
    cosT = np.cos(emb).T  # [HD, S]
    sinT = np.sin(emb).T

    h2 = HD // 2
    rotmT = np.zeros((P, P), np.float32)
    for p in range(h2):
        rotmT[p + h2, p] = -1.0
    for p in range(h2, HD):
        rotmT[p - h2, p] = 1.0
    rotm = rotmT.astype(bf16)

    scale = 1.0 / math.sqrt(HD)
    gq_a = np.ascontiguousarray(
        (np.asarray(q_gamma, np.float64) * scale).reshape(NH, P).T
    ).astype(np.float32)
    bq_a = np.ascontiguousarray(
        (np.asarray(q_beta, np.float64) * scale).reshape(NH, P).T
    ).astype(np.float32)
    gk_a = np.ascontiguousarray(
        np.asarray(k_gamma, np.float32).reshape(NH, P).T
    )
    bk_a = np.ascontiguousarray(
        np.asarray(k_beta, np.float32).reshape(NH, P).T
    )
    onesc = np.ones((P, 1), np.float32)
    onesr = np.ones((1, P), np.float32)

    xb_T = {}
    in_maps = []
    meta = []
    for c in range(n_cores):
        b = c // 2
        r = c % 2
        perm = _PERMS[r]
        ptok = np.concatenate(
            [np.arange(pb * QT, (pb + 1) * QT) for pb in perm]
        )
        if b not in xb_T:
            xb_T[b] = np.ascontiguousarray(
                np.asarray(x[b], np.float32).T
            )  # [D, S] f32
        xT = np.ascontiguousarray(xb_T[b][:, ptok]).astype(bf16)
        cosp = np.ascontiguousarray(cosT[:, ptok]).astype(bf16)
        sinp = np.ascontiguousarray(sinT[:, ptok]).astype(bf16)

        # masks in PERMUTED kv space; q slabs at permuted positions Q_POS.
        # Elementwise masks only on diagonal slots; other slots use the
        # per-row exp bias: -EXP_BIAS for fully valid rows, BIAS_INVALID
        # for fully invalid rows.
        masks = np.zeros([NQ, P, MAXM, QT], np.float32)
        biast = np.full([P, NQ, 16], -EXP_BIAS, np.float32)
        for t in range(NQ):
            gq_tok = ptok[Q_POS[t] * QT + np.arange(QT)]
            gq_max = gq_tok.max()
            for mi, kc in enumerate(MASKED[t]):
                gkv = ptok[kc * P + np.arange(P)]
                masks[t, :, mi, :] = (
                    gkv[:, None] <= gq_tok[None, :]
                ).astype(np.float32)
            for kc in range(16):
                gkv = ptok[kc * P + np.arange(P)]
                biast[:, t, kc] = np.where(gkv <= gq_max, -EXP_BIAS,
                                           BIAS_INVALID)
        masks = masks.astype(bf16)

        qtok = np.concatenate(
            [np.arange(perm[pq] * QT, (perm[pq] + 1) * QT) for pq in Q_POS]
        )
        in_maps.append(dict(
            xT=xT, wqk=wqk_t, wv=wv_t, wo=wo_t,
            cos=cosp, sin=sinp,
            gq=gq_a, bq=bq_a, gk=gk_a, bk=bk_a, masks=masks,
            biast=biast, onesc=onesc, onesr=onesr, rotm=rotm,
        ))
        meta.append(dict(b=b, qtok=qtok))
    return in_maps, meta


_PROGRAM_CACHE = {}


def _get_program():
    if "full" not in _PROGRAM_CACHE:
        _PROGRAM_CACHE["full"] = build_program()
    return _PROGRAM_CACHE["full"]


def run_full(x, w_in, w_out, q_gamma, q_beta, k_gamma, k_beta,
             trace=False):
    from concourse.bass_utils import run_bass_kernel_spmd

    B = x.shape[0]
    n_cores = 2 * B
    in_maps, meta = make_host_data(
        x, w_in, w_out, q_gamma, q_beta, k_gamma, k_beta,
    )
    nc = _get_program()
    res = run_bass_kernel_spmd(
        nc, in_maps, core_ids=list(range(n_cores)), trace=trace,
    )
    out = np.empty((B, S, D), np.float32)
    for c in range(n_cores):
        o = res.results[c]["out"]  # [D, NQTOK]
        out[meta[c]["b"], meta[c]["qtok"], :] = o.T
    return out, res


def kernel(x, w_in, w_out, q_gamma, q_beta, k_gamma, k_beta, n_heads=16,
           **_ignored):
    x = np.asarray(x, np.float32)
    assert int(np.asarray(n_heads)) * HD == x.shape[-1]
    out, _ = run_full(
        np.asarray(x, np.float32),
        np.asarray(w_in, np.float32),
        np.asarray(w_out, np.float32),
        np.asarray(q_gamma, np.float32),
        np.asarray(q_beta, np.float32),
        np.asarray(k_gamma, np.float32),
        np.asarray(k_beta, np.float32),
    )
    return out


# revision 31
# speedup vs baseline: 1.5956x; 1.0057x over previous
"""Trainium2 Bass kernel for a custom attention block (qkv-proj + LN(q,k) +
RoPE + causal attention + out-proj), distributed over 8 NeuronCores.

Sharding: 2 cores per batch (B=4). Core role r=c%2 takes q-token blocks
{0,3} (r=0) or {1,2} (r=1) of 512 tokens; every core computes K/V for the
full 2048-token sequence of its batch (no collectives). The compiled
program is identical on all cores; per-core differences are input data
only. To keep the q-slab offsets compile-time-constant, each core sees
the sequence in a per-role BLOCK PERMUTATION (r=0: 0,1,2,3; r=1:
1,0,3,2), so its q blocks always sit at permuted positions {0,3} and the
causal diagonal lands on the same slot indices for both roles. The
cos/sin tables, causal masks, exp row-biases and output assembly are
permutation-aware host data.

All matmuls run in bf16 (same PE rate as fp32r but faster weight loads,
half the DMA/SBUF), with fp32 PSUM accumulation. x is SBUF-resident; q
stays SBUF-resident post-rope; k round-trips through DRAM feature-major;
v is computed feature-major (so the PE reuses each stationary weight
tile across 4 moving tiles) and transposed to token-major on the fly by
DMA-transpose loads during attention.

Engine split: PE does projections/scores/PV; Scalar does exp, squares
and PSUM->SBUF copies; DVE does LN/rope muls, masks and half the
softmax-denominator accumulation; GpSimd does the other half plus the
rope add and LN sumsq chains. Softmax denominator = chained elementwise
adds of the exp tiles + one ones-matmul partition reduction per head.
"""

import math

import numpy as np

import concourse.bass as bass
import concourse.mybir as mybir
import concourse.tile as tile
from concourse import bacc
from concourse.bass import ds

F32 = mybir.dt.float32
F32R = mybir.dt.float32r
BF16 = mybir.dt.bfloat16
AF = mybir.ActivationFunctionType
OP = mybir.AluOpType

P = 128
HD = 128
D = 2048
S = 2048
NH = D // HD          # 16 heads = feature chunks
DC = D // P           # 16 contraction chunks
NQTOK = 1024          # q tokens per core
QT = 512              # q/attention tile width (moving dim)
NQ = NQTOK // QT      # 2 q tiles per core
EXP_BIAS = 8.0
EPS = 1e-5
SLOTS = (8, 16)       # kv 128-chunks per q tile (max over the two roles)
# elementwise masks only on the diagonal slots (identical for both roles
# thanks to the block permutation); everything else is handled by the
# per-row exp bias (-EXP_BIAS valid / BIAS_INVALID invalid).
MASKED = ((0, 1, 2, 3), (12, 13, 14, 15))
MAXM = 4
BIAS_INVALID = -30.0
Q_POS = (0, 3)        # structural (permuted) block positions of q slabs
KC = S // P           # 16 kv chunks
LOOKAHEAD = 2         # attention score-slot software pipeline depth


def _r(ap):
    """fp32 -> fp32r view for matmul operands."""
    return ap.bitcast(F32R)


def _v3(ap):
    """[P, n*128] AP -> [P, n, 128] view (avoids 1-free-dim DMA splits)."""
    return ap.rearrange("p (a x) -> p a x", x=P)


def build_program():
    nc = bacc.Bacc("TRN2", target_bir_lowering=False, debug=False)

    # ---- I/O ----
    xT_i = nc.dram_tensor("xT", [D, S], BF16, kind="ExternalInput").ap()
    wqk_i = nc.dram_tensor("wqk", [2 * NH, P, DC, P], BF16,
                           kind="ExternalInput").ap()
    wv_i = nc.dram_tensor("wv", [DC, P, NH, P], BF16,
                          kind="ExternalInput").ap()
    wo_i = nc.dram_tensor("wo", [NH, P, NH, P], BF16,
                          kind="ExternalInput").ap()
    cos_i = nc.dram_tensor("cos", [HD, S], BF16, kind="ExternalInput").ap()
    sin_i = nc.dram_tensor("sin", [HD, S], BF16, kind="ExternalInput").ap()
    gq_i = nc.dram_tensor("gq", [P, NH], F32, kind="ExternalInput").ap()
    bq_i = nc.dram_tensor("bq", [P, NH], F32, kind="ExternalInput").ap()
    gk_i = nc.dram_tensor("gk", [P, NH], F32, kind="ExternalInput").ap()
    bk_i = nc.dram_tensor("bk", [P, NH], F32, kind="ExternalInput").ap()
    masks_i = nc.dram_tensor("masks", [NQ, P, MAXM, QT], BF16,
                             kind="ExternalInput").ap()
    biast_i = nc.dram_tensor("biast", [P, NQ, 16], F32,
                             kind="ExternalInput").ap()
    onesc_i = nc.dram_tensor("onesc", [P, 1], F32, kind="ExternalInput").ap()
    onesr_i = nc.dram_tensor("onesr", [1, P], F32, kind="ExternalInput").ap()
    rotm_i = nc.dram_tensor("rotm", [P, P], BF16, kind="ExternalInput").ap()
    out_t = nc.dram_tensor("out", [D, NQTOK], F32, kind="ExternalOutput").ap()

    with tile.TileContext(nc) as tc:
        import contextlib

        ctx = contextlib.ExitStack()
        with ctx:
            sb = ctx.enter_context(tc.tile_pool(name="sb", bufs=1))
            psum = ctx.enter_context(tc.tile_pool(name="ps", bufs=1, space="PSUM"))
            dram = ctx.enter_context(tc.tile_pool(name="dram", bufs=1, space="DRAM"))

            # ---- DRAM scratch ----
            kts = dram.tile([P, NH, S], BF16, tag="kts", name="kts")
            vT = dram.tile([D, S], BF16, tag="vT", name="vT")

            # ---- constants / small inputs ----
            ones_col = sb.tile([P, 1], F32, tag="ones_col", name="ones_col")
            nc.sync.dma_start(_r(ones_col), _r(onesc_i))
            ones_row = sb.tile([1, P], F32, tag="ones_row", name="ones_row")
            nc.sync.dma_start(_r(ones_row), _r(onesr_i))
            eps1 = sb.tile([1, 1], F32, tag="eps1", name="eps1")
            nc.vector.memset(eps1, EPS)
            zero1 = sb.tile([1, 1], F32, tag="zero1", name="zero1")
            nc.vector.memset(zero1, 0.0)
            biast = sb.tile([P, NQ, 16], F32, tag="biast", name="biast")
            nc.sync.dma_start(biast, biast_i)
            rotm = sb.tile([P, P], BF16, tag="rotm", name="rotm")
            nc.sync.dma_start(rotm, rotm_i)
            gq = sb.tile([P, NH], F32, tag="gq", name="gq")
            nc.sync.dma_start(gq, gq_i)
            bq = sb.tile([P, NH], F32, tag="bq", name="bq")
            nc.sync.dma_start(bq, bq_i)
            gk = sb.tile([P, NH], F32, tag="gk", name="gk")
            nc.sync.dma_start(gk, gk_i)
            bk = sb.tile([P, NH], F32, tag="bk", name="bk")
            nc.sync.dma_start(bk, bk_i)
            cos_t = sb.tile([HD, S], BF16, tag="cos_t", name="cos_t")
            nc.sync.dma_start(_v3(cos_t), _v3(cos_i))
            sin_t = sb.tile([HD, S], BF16, tag="sin_t", name="sin_t")
            nc.sync.dma_start(_v3(sin_t), _v3(sin_i))

            # ---- resident x: [128, DC, S] bf16 (64KB/partition) ----
            xsb = sb.tile([P, DC, S], BF16, tag="xsb", bufs=1, name="xsb")
            for d in range(DC):
                nc.sync.dma_start(xsb[:, d], xT_i[ds(d * P, P), :])

            # ---- resident q (post-LN+rope): [128, NH, NQTOK] bf16 ----
            q_res = sb.tile([P, NH, NQTOK], BF16, tag="q_res", bufs=1,
                            name="q_res")

            def proj_group(ec_base, slab_offs, q_dst_offs, g_sb, b_sb):
                """Project x -> feature-partition [128, QT] tiles for each
                token slab in this group. Emits the matmul phase and
                returns a closure that emits the LN + rope tail (so the
                caller can interleave it into the next group's matmul
                stream and keep the PE dense).

                slab_offs: compile-time token offsets into the permuted
                sequence (index into x and cos/sin). If q_dst_offs is not
                None the result lands at q_res[:, :, q_dst_off]; else it
                DMAs to kts[:, :, slab_off].
                """
                n_s = len(slab_offs)
                holds = []
                for i in range(n_s):
                    if q_dst_offs is not None:
                        holds.append(q_res[:, :, ds(q_dst_offs[i], QT)])
                    else:
                        h = sb.tile([P, NH, QT], BF16, tag="khold", bufs=2,
                                    name="khold")
                        holds.append(h)
                sqsums = []
                for i in range(n_s):
                    sqsums.append(sb.tile([P, QT], F32, tag="acc", bufs=3,
                                          name="sqsum"))
                for ec in range(NH):
                    w = sb.tile([P, DC, P], BF16, tag="w", bufs=2, name="w")
                    nc.sync.dma_start(w, wqk_i[ec_base + ec])
                    pss = [psum.tile([P, QT], F32, tag="mm", bufs=5, name="ps")
                           for _ in range(n_s)]
                    for d in range(DC):
                        for i in range(n_s):
                            nc.tensor.matmul(
                                pss[i],
                                lhsT=w[:, d],
                                rhs=xsb[:, d, ds(slab_offs[i], QT)],
                                start=(d == 0),
                                stop=(d == DC - 1),
                            )
                    for i in range(n_s):
                        nc.scalar.copy(holds[i][:, ec], pss[i])
                        sq = sb.tile([P, QT], BF16, tag="sq", bufs=2,
                                     name="sq")
                        nc.scalar.square(sq, pss[i])
                        if ec == 0:
                            nc.vector.tensor_copy(_r(sqsums[i]), sq)
                        else:
                            nc.gpsimd.tensor_tensor(_r(sqsums[i]), sqsums[i],
                                                    sq, op=OP.add)

                def tail():
                    for i in range(n_s):
                        hold = holds[i]
                        csl = ds(slab_offs[i], QT)
                        # per-token sumsq: partition-sum via ones-matmul
                        pstat = psum.tile([1, QT], F32, tag="stat", bufs=3,
                                          name="pstat")
                        nc.tensor.matmul(pstat, lhsT=_r(ones_col),
                                         rhs=_r(sqsums[i]))
                        # rsig = exp(-0.5 * ln(sumsq/D + eps))
                        lnv = sb.tile([1, QT], F32, tag="stats_sb", bufs=4,
                                      name="lnv")
                        nc.scalar.activation(lnv, pstat, AF.Ln,
                                             scale=1.0 / D, bias=eps1)
                        rsig = sb.tile([1, QT], F32, tag="stats_sb", bufs=4,
                                       name="rsig")
                        nc.scalar.activation(_r(rsig), lnv, AF.Exp,
                                             bias=zero1, scale=-0.5)
                        ps_rep = psum.tile([P, QT], F32, tag="mm", bufs=5,
                                           name="ps_rep")
                        nc.tensor.matmul(ps_rep, lhsT=_r(ones_row),
                                         rhs=_r(rsig))
                        # pass 1: LN apply on all chunks (DVE)
                        for ec in range(NH):
                            ch = hold[:, ec]
                            nc.vector.tensor_tensor(ch, ch, ps_rep,
                                                    op=OP.mult)
                            nc.vector.tensor_scalar(
                                ch, ch,
                                scalar1=g_sb[:, ds(ec, 1)],
                                scalar2=b_sb[:, ds(ec, 1)],
                                op0=OP.mult, op1=OP.add,
                            )
                        # pass 2: rope; rotation matmuls stream back-to-back
                        for ec in range(NH):
                            ch = hold[:, ec]
                            ps_rot = psum.tile([P, QT], F32, tag="mm",
                                               bufs=5, name="ps_rot")
                            nc.tensor.matmul(ps_rot, lhsT=rotm, rhs=ch)
                            tmp = sb.tile([P, QT], BF16, tag="rtmp", bufs=2,
                                          name="rtmp")
                            nc.vector.tensor_tensor(tmp, ps_rot,
                                                    sin_t[:, csl], op=OP.mult)
                            nc.vector.tensor_tensor(ch, ch, cos_t[:, csl],
                                                    op=OP.mult)
                            nc.gpsimd.tensor_tensor(ch, ch, tmp, op=OP.add)
                        if q_dst_offs is None:
                            nc.sync.dma_start(
                                kts[:, :, ds(slab_offs[i], QT)], hold
                            )

                return tail

            def v_chunk(f):
                """Phase V chunk: w-stationary (reused across 4 moving
                tiles), writes v^T feature-major to DRAM."""
                wvf = sb.tile([P, DC, P], BF16, tag="w", bufs=2, name="wvf")
                nc.sync.dma_start(wvf, wv_i[:, :, f, :].rearrange(
                    "d p j -> p d j"))
                psv = [psum.tile([P, QT], F32, tag="mm", bufs=5, name="psv")
                       for _ in range(4)]
                for d in range(DC):
                    for ts in range(4):
                        nc.tensor.matmul(
                            psv[ts],
                            lhsT=wvf[:, d],
                            rhs=xsb[:, d, ds(ts * QT, QT)],
                            start=(d == 0),
                            stop=(d == DC - 1),
                        )
                for ts in range(4):
                    vtsb = sb.tile([P, QT], BF16, tag="vsb", bufs=3,
                                   name="vtsb")
                    nc.scalar.copy(vtsb, psv[ts])
                    nc.gpsimd.dma_start(
                        _v3(vT[ds(f * P, P), ds(ts * QT, QT)]), _v3(vtsb)
                    )

            # ---- Projections: each group's LN/rope tail is emitted
            # inside the NEXT group's matmul stream so the PE stays dense.
            tail_q = proj_group(0, [Q_POS[0] * QT, Q_POS[1] * QT], [0, QT],
                                gq, bq)
            pending_tail = tail_q
            for g in range(4):
                t_k = proj_group(NH, [g * QT], None, gk, bk)
                pending_tail()
                pending_tail = t_k
            v_chunk(0)
            pending_tail()
            for f in range(1, NH):
                v_chunk(f)

            # ---- Attention + out-projection per q tile ----
            for t in range(NQ):
                qsl_off = t * QT
                n_slots = SLOTS[t]
                # masks overlay the (now dead) cos buffer
                mt = sb.tile([P, MAXM, QT], BF16, tag="cos_t", bufs=1,
                             name="mt")
                nc.sync.dma_start(mt, masks_i[t])
                mpos = {kc: i for i, kc in enumerate(MASKED[t])}
                ot_res = sb.tile([P, NH, QT], BF16, tag="khold", bufs=2,
                                 name="ot_res")
                pending = None

                def finish_norm(pending):
                    psout_p, esum_p, h_p = pending
                    psden = psum.tile([1, QT], F32, tag="stat", bufs=3,
                                      name="psden")
                    nc.tensor.matmul(psden, lhsT=_r(ones_col), rhs=_r(esum_p))
                    rec0 = sb.tile([1, QT], F32, tag="stats_sb", bufs=4,
                                   name="rec0")
                    with nc.allow_low_precision(
                        reason="denominator reciprocal, 18 bits is plenty"
                    ):
                        nc.vector.reciprocal_approx_fast(rec0, psden)
                    rec = sb.tile([1, QT], F32, tag="stats_sb", bufs=4,
                                  name="rec")
                    nc.vector.tensor_copy(_r(rec), rec0)
                    psr = psum.tile([P, QT], F32, tag="mm", bufs=5,
                                    name="psr")
                    nc.tensor.matmul(psr, lhsT=_r(ones_row), rhs=_r(rec))
                    nc.vector.tensor_copy(ot_res[:, h_p], psout_p)
                    nc.vector.tensor_tensor(ot_res[:, h_p], ot_res[:, h_p],
                                            psr, op=OP.mult)

                for h in range(NH):
                    ksl = sb.tile([P, KC, P], BF16, tag="kslab", bufs=2,
                                  name="ksl")
                    nc.sync.dma_start(
                        ksl[:, ds(0, n_slots)],
                        kts[:, h].rearrange("p (c x) -> p c x", x=P)[
                            :, ds(0, n_slots)],
                    )
                    vsl = sb.tile([P, KC, HD], BF16, tag="vslab", bufs=2,
                                  name="vsl")
                    nc.sync.dma_start_transpose(
                        vsl[:, ds(0, n_slots)],
                        vT[ds(h * HD, HD), ds(0, n_slots * P)],
                    )
                    psout = psum.tile([P, QT], F32, tag="mm", bufs=5,
                                      name="psout")
                    esum = sb.tile([P, QT], F32, tag="acc", bufs=3,
                                   name="esum")
                    esum_b = sb.tile([P, QT], F32, tag="sin_t", bufs=1,
                                     name="esum_b")
                    qsl = q_res[:, h, ds(qsl_off, QT)]

                    ets = {}

                    def emit_score(s):
                        pss = psum.tile([P, QT], F32, tag="mm", bufs=5,
                                        name="pss")
                        nc.tensor.matmul(pss, lhsT=ksl[:, s], rhs=qsl)
                        et = sb.tile([P, QT], BF16, tag="exp", bufs=4,
                                     name="et")
                        nc.scalar.activation(et, pss, AF.Exp,
                                             bias=biast[:, t, ds(s, 1)])
                        if s in mpos:
                            nc.vector.tensor_tensor(et, et, mt[:, mpos[s]],
                                                    op=OP.mult)
                        ets[s] = et

                    for s in range(min(LOOKAHEAD, n_slots)):
                        emit_score(s)
                    # previous head's normalization, pipelined behind our
                    # prologue so the PE never waits on its denominator
                    if pending is not None:
                        finish_norm(pending)
                    for s in range(n_slots):
                        if s + LOOKAHEAD < n_slots:
                            emit_score(s + LOOKAHEAD)
                        et = ets.pop(s)
                        nc.tensor.matmul(
                            psout,
                            lhsT=vsl[:, s],
                            rhs=et,
                            start=(s == 0),
                            stop=(s == n_slots - 1),
                        )
                        # denominator accumulation off the PE: two chains
                        # partitioned in time — GpSimd takes the early
                        # slots, DVE the late ones (less SBUF-port overlap)
                        half = n_slots // 2
                        if s == 0:
                            nc.vector.tensor_copy(_r(esum_b), et)
                        elif s < half:
                            nc.gpsimd.tensor_tensor(_r(esum_b), esum_b, et,
                                                    op=OP.add)
                        elif s == half:
                            nc.vector.tensor_copy(_r(esum), et)
                        else:
                            nc.vector.tensor_tensor(_r(esum), esum, et,
                                                    op=OP.add)
                    nc.vector.tensor_tensor(_r(esum), esum, esum_b, op=OP.add)
                    pending = (psout, esum, h)
                finish_norm(pending)

                # ---- out-projection for this q tile ----
                for e in range(NH):
                    wot = sb.tile([P, NH, P], BF16, tag="wot", bufs=2,
                                  name="wot")
                    nc.sync.dma_start(wot, wo_i[e])
                    psf = psum.tile([P, QT], F32, tag="mm", bufs=5,
                                    name="psf")
                    for h in range(NH):
                        nc.tensor.matmul(
                            psf,
                            lhsT=wot[:, h],
                            rhs=ot_res[:, h],
                            start=(h == 0),
                            stop=(h == NH - 1),
                        )
                    fsb = sb.tile([P, QT], F32, tag="fsb", bufs=2,
                                  name="fsb")
                    nc.scalar.copy(fsb, psf)
                    nc.sync.dma_start(
                        _v3(out_t[ds(e * P, P), ds(qsl_off, QT)]), _v3(fsb)
                    )

    nc.compile()
    return nc


# --------------------------------------------------------------------------
# Host-side prep and driver
# --------------------------------------------------------------------------

_PERMS = {0: (0, 1, 2, 3), 1: (1, 0, 3, 2)}


def make_host_data(x, w_in, w_out, q_gamma, q_beta, k_gamma, k_beta):
    """Build per-core in_maps (list of dicts) + assembly metadata."""
    import ml_dtypes
    bf16 = ml_dtypes.bfloat16

    B = x.shape[0]
    n_cores = 2 * B

    w64 = np.asarray(w_in, np.float64)
    wq = w64[0:D]
    wk = w64[D:2 * D]
    wv = w64[2 * D:3 * D]
    wq_c = wq - wq.mean(axis=0, keepdims=True)
    wk_c = wk - wk.mean(axis=0, keepdims=True)
    wqkT2 = np.concatenate([wq_c.T, wk_c.T], axis=1)   # [D, 2D]
    wqk_t = np.ascontiguousarray(
        wqkT2.reshape(DC, P, 2 * NH, P).transpose(2, 1, 0, 3)
    ).astype(bf16)
    wvT = wv.T  # [D(d), D(f)]
    wv_t = np.ascontiguousarray(
        wvT.reshape(DC, P, NH, P)
    ).astype(bf16)
    woT = np.asarray(w_out, np.float64).T  # [D(hfeat), D(eout)]
    wo_t = np.ascontiguousarray(
        woT.reshape(NH, P, NH, P).transpose(2, 1, 0, 3)
    ).astype(bf16)

    inv = 1.0 / (10000.0 ** (np.arange(0, HD, 2, dtype=np.float64) / HD))
    tpos = np.arange(S, dtype=np.float64)
    fr = np.outer(tpos, inv)
    emb = np.concatenate([fr, fr], axis=-1)  # [S, HD]
    cosT = np.cos(emb).T  # [HD, S]
    sinT = np.sin(emb).T

    h2 = HD // 2
    rotmT = np.zeros((P, P), np.float32)
    for p in range(h2):
        rotmT[p + h2, p] = -1.0
    for p in range(h2, HD):
        rotmT[p - h2, p] = 1.0
    rotm = rotmT.astype(bf16)

    scale = 1.0 / math.sqrt(HD)
    gq_a = np.ascontiguousarray(
        (np.asarray(q_gamma, np.float64) * scale).reshape(NH, P).T
    ).astype(np.float32)
    bq_a = np.ascontiguousarray(
        (np.asarray(q_beta, np.float64) * scale).reshape(NH, P).T
    ).astype(np.float32)
    gk_a = np.ascontiguousarray(
        np.asarray(k_gamma, np.float32).reshape(NH, P).T
    )
    bk_a = np.ascontiguousarray(
        np.asarray(k_beta, np.float32).reshape(NH, P).T
    )
    onesc = np.ones((P, 1), np.float32)
    onesr = np.ones((1, P), np.float32)

    xb_T = {}
    in_maps = []
    meta = []
    for c in range(n_cores):
        b = c // 2
        r = c % 2
        perm = _PERMS[r]
        ptok = np.concatenate(
            [np.arange(pb * QT, (pb + 1) * QT) for pb in perm]
        )
        if b not in xb_T:
            xb_T[b] = np.ascontiguousarray(
                np.asarray(x[b], np.float32).T
            )  # [D, S] f32
        xT = np.ascontiguousarray(xb_T[b][:, ptok]).astype(bf16)
        cosp = np.ascontiguousarray(cosT[:, ptok]).astype(bf16)
        sinp = np.ascontiguousarray(sinT[:, ptok]).astype(bf16)

        # masks in PERMUTED kv space; q slabs at permuted positions Q_POS.
        # Elementwise masks only on diagonal slots; other slots use the
        # per-row exp bias: -EXP_BIAS for fully valid rows, BIAS_INVALID
        # for fully invalid rows.
        masks = np.zeros([NQ, P, MAXM, QT], np.float32)
        biast = np.full([P, NQ, 16], -EXP_BIAS, np.float32)
        for t in range(NQ):
            gq_tok = ptok[Q_POS[t] * QT + np.arange(QT)]
            gq_max = gq_tok.max()
            for mi, kc in enumerate(MASKED[t]):
                gkv = ptok[kc * P + np.arange(P)]
                masks[t, :, mi, :] = (
                    gkv[:, None] <= gq_tok[None, :]
                ).astype(np.float32)
            for kc in range(16):
                gkv = ptok[kc * P + np.arange(P)]
                biast[:, t, kc] = np.where(gkv <= gq_max, -EXP_BIAS,
                                           BIAS_INVALID)
        masks = masks.astype(bf16)

        qtok = np.concatenate(
            [np.arange(perm[pq] * QT, (perm[pq] + 1) * QT) for pq in Q_POS]
        )
        in_maps.append(dict(
            xT=xT, wqk=wqk_t, wv=wv_t, wo=wo_t,
            cos=cosp, sin=sinp,
            gq=gq_a, bq=bq_a, gk=gk_a, bk=bk_a, masks=masks,
            biast=biast, onesc=onesc, onesr=onesr, rotm=rotm,
        ))
        meta.append(dict(b=b, qtok=qtok))
    return in_maps, meta


_PROGRAM_CACHE = {}


def _get_program():
    if "full" not in _PROGRAM_CACHE:
        _PROGRAM_CACHE["full"] = build_program()
    return _PROGRAM_CACHE["full"]


def run_full(x, w_in, w_out, q_gamma, q_beta, k_gamma, k_beta,
             trace=False):
    from concourse.bass_utils import run_bass_kernel_spmd

    B = x.shape[0]
    n_cores = 2 * B
    in_maps, meta = make_host_data(
        x, w_in, w_out, q_gamma, q_beta, k_gamma, k_beta,
    )
    nc = _get_program()
    res = run_bass_kernel_spmd(
        nc, in_maps, core_ids=list(range(n_cores)), trace=trace,
    )
    out = np.empty((B, S, D), np.float32)
    for c in range(n_cores):
        o = res.results[c]["out"]  # [D, NQTOK]
        out[meta[c]["b"], meta[c]["qtok"], :] = o.T
    return out, res


def kernel(x, w_in, w_out, q_gamma, q_beta, k_gamma, k_beta, n_heads=16,
           **_ignored):
    x = np.asarray(x, np.float32)
    assert int(np.asarray(n_heads)) * HD == x.shape[-1]
    out, _ = run_full(
        np.asarray(x, np.float32),
        np.asarray(w_in, np.float32),
        np.asarray(w_out, np.float32),
        np.asarray(q_gamma, np.float32),
        np.asarray(q_beta, np.float32),
        np.asarray(k_gamma, np.float32),
        np.asarray(k_beta, np.float32),
    )
    return out
